# revision 8
# baseline (speedup 1.0000x reference)
"""Bass/Tile program for nn_DTWModel on TRN2: conv encoders + euclidean dist
+ global min-max norm + exact DTW (forward wavefront row-scans, bulk choice
extraction, backward path-marking wavefront).

Layout summary (per core, one sample):
- rows r=0..511 of the DTW matrix; partition p owns rows 4p..4p+3.
- forward: unit (k,s) = (row 4p+k, col-strip s of width W=32) processed at
  step m = 4p + 4s + k.  All partitions share step-uniform APs via a
  32-slot rotating window (2 group tiles of 16 slots); slot = m % 32.
- CB slot layout: [guard][32 cost values]; guard(slot m) = last value of
  slot m-4 (same row, previous strip) = cost[r][s*W-1].
- scan: state = (u min state) + d  == min(min(pd,up),left)+d of reference.
- u = min(CB(m-1)[0:32], CB(m-1)[1:33]) = min(pd, up) from row r-1.
- k=0 rows need row 4p-1 from partition p-1: PE matmul with shifted
  identity moves the slot down one partition (psum[p] = slot[p-1]).
- cost deskewed to DRAM via p-linear strided DMAs every 16 steps.
- bulk phase recomputes choices C from cost with reference tie-break, then
  static masks E0s/E2s/c1s and seed Sd, all written to padded DRAM.
- backward: P[i][j] = max(Sd, E0s*P[i+1][j+1], E2s*P[i+1][j], c1s-scan)
  processed as mirrored wavefront with reversed ttscan; P masked NaN-proof
  by validity mask M via (P*M) is_ge 0.5.
"""
import sys as _sys
if '/opt/trn_rl_repo' not in _sys.path:
    _sys.path.insert(0, '/opt/trn_rl_repo')
import numpy as np
import concourse.bass as bass
import concourse.mybir as mybir
from concourse.vector_clock import ScopedClock
from concourse.tile import TileContext

F32 = mybir.dt.float32
I32 = mybir.dt.int32
OP = mybir.AluOpType
ACT = mybir.ActivationFunctionType
AX = mybir.AxisListType

LARGE = float(np.float32(1e30))
SLOPE = float(np.float32(0.2))
DEBUG = False
STOP_AFTER = None  # 'front'|'fwd'|'bulk'|None

W = 32          # strip width
U = 33          # slot width (guard + W)
S = 512 // W    # strips per row = 16
NSTEP = 4 * 127 + 4 * (S - 1) + 3 + 1   # 572 steps, m in [0, 572)
ROUND = 32
NROUND = (NSTEP + ROUND - 1) // ROUND
GW = ROUND * U  # group tile width = 528

# cost_pad DRAM layout
CS = 4672       # row stride (cols)
CO = 4064       # data col offset; col CO-1 = INF guard (j=-1)
CROWS = 514     # row i stored at row i+1; row 0 = INF

# C_pad layout: row r stored at r+1; rows 0 unused, row 513 = 3.0 (virtual r=512)
CPR, CPC = 515, 520

# E/Sd/P pads
EC = 8672
CO_E = 4096
EROWS = 512


class SplitDrainTileContext(TileContext):
    """Final drain must carry <=1 sem wait for this neuronxcc."""

    def _drain_and_barrier(self, tick_clock, wait_clock):
        drain_inst = self.nc.sync.drain()
        wait_clock.add_sem_waits(
            drain_inst.ins, ScopedClock({None: tick_clock.global_clock})
        )
        si = drain_inst.ins.sync_info
        waits = list(si.on_wait or [])
        if len(waits) > 1:
            si.on_wait[:] = waits[:1]
            for w_ in waits[1:]:
                nop = self.nc.sync.nop(nofuse=True, hint="split_drain_wait")
                nsi = nop.ins.sync_info
                if nsi is None:
                    nop.ins.sync_info = mybir.SyncInfo(on_wait=[w_], on_update=[])
                else:
                    nsi.on_wait.append(w_)
        self.nc.all_engine_barrier()
        assert self.sems is not None
        popped = self.nc._tile_sem_poison_stack.pop()
        assert popped is self._sem_poison
        self.nc.clear_and_free_semaphores(list(self.sems.allocated().values()))
        self.nc.all_engine_barrier()


def rap(t, offset, ap):
    return bass.AP(tensor=t[:].tensor, offset=int(offset), ap=[[int(a), int(b)] for a, b in ap])


def build_program(n_cores=8, with_collective=True):
    nc = bass.Bass("TRN2", target_bir_lowering=False, debug=False,
                   num_devices=n_cores)

    # ---------------- dram tensors ----------------
    din = {}
    din['vec'] = nc.dram_tensor("vec", [126, 512], F32, kind="ExternalInput")
    din['music'] = nc.dram_tensor("music", [80, 512], F32, kind="ExternalInput")
    din['rl'] = nc.dram_tensor("rl", [1], I32, kind="ExternalInput")
    wspec = [('q1', 126, 126), ('q2', 126, 128), ('q3', 128, 128),
             ('k1', 80, 80), ('k2', 80, 128), ('k3', 128, 128)]
    for nm, ci, co in wspec:
        din['w' + nm] = nc.dram_tensor("w" + nm, [3, ci, co], F32, kind="ExternalInput")
        din['b' + nm] = nc.dram_tensor("b" + nm, [co, 1], F32, kind="ExternalInput")
    din['eye_up'] = nc.dram_tensor("eye_up", [128, 128], F32, kind="ExternalInput")
    din['eye_dn'] = nc.dram_tensor("eye_dn", [128, 128], F32, kind="ExternalInput")
    din['ones_bc'] = nc.dram_tensor("ones_bc", [1, 128], F32, kind="ExternalInput")

    dist_out = nc.dram_tensor("dist", [512, 512], F32, kind="ExternalOutput")
    path_out = nc.dram_tensor("path01", [512, 512], F32, kind="ExternalOutput")

    cost_pad = nc.dram_tensor("cost_pad", [CROWS * CS], F32)
    c_pad = nc.dram_tensor("c_pad", [CPR * CPC], F32)
    e0_pad = nc.dram_tensor("e0_pad", [EROWS * EC], F32)
    e2_pad = nc.dram_tensor("e2_pad", [EROWS * EC], F32)
    c1_pad = nc.dram_tensor("c1_pad", [EROWS * EC], F32)
    sd_pad = nc.dram_tensor("sd_pad", [EROWS * EC], F32)
    p_pad = nc.dram_tensor("p_pad", [EROWS * EC], F32)
    d_stage = nc.dram_tensor("d_stage", [524 * 512], F32)

    dbg = {}
    if DEBUG:
        dbg['qlat'] = nc.dram_tensor("dbg_qlat", [128, 512], F32, kind="ExternalOutput")
        dbg['klat'] = nc.dram_tensor("dbg_klat", [128, 512], F32, kind="ExternalOutput")
        dbg['cost'] = nc.dram_tensor("dbg_cost", [512, 512], F32, kind="ExternalOutput")
        dbg['C'] = nc.dram_tensor("dbg_C", [512, 512], F32, kind="ExternalOutput")

    with SplitDrainTileContext(nc) as tc:
        _build_body(nc, tc, din, dist_out, path_out, cost_pad, c_pad,
                    e0_pad, e2_pad, c1_pad, sd_pad, p_pad, d_stage,
                    with_collective, n_cores, dbg)
    _split_multi_waits(nc)
    return nc


def _split_multi_waits(nc, max_waits=1):
    """This neuronxcc rejects instructions with more than ~1-2 sync waits.
    Move extra waits onto same-engine NoOps inserted just before."""
    import bass_rust as _br
    ctr = [0]
    for f in nc.m.functions:
        for bb in f.blocks:
            newlist = []
            for inst in bb.instructions:
                si = inst.sync_info
                waits = list(si.on_wait) if (si and si.on_wait) else []
                if len(waits) > max_waits:
                    keep = waits[:max_waits]
                    extra = waits[max_waits:]
                    si.on_wait[:] = keep
                    for w_ in extra:
                        ctr[0] += 1
                        nop = _br.InstNoOp(name=f"waitsplit_{ctr[0]}")
                        nop.engine = inst.engine
                        nop.sync_info = mybir.SyncInfo(on_wait=[w_], on_update=[])
                        nc.register_instruction(nop, overwrite=True)
                        newlist.append(nop)
                newlist.append(inst)
            if ctr[0]:
                bb.instructions[:] = newlist
    return ctr[0]


def _build_body(nc, tc, din, dist_out, path_out, cost_pad, c_pad,
                e0_pad, e2_pad, c1_pad, sd_pad, p_pad, d_stage, with_collective,
                n_cores, dbg):
    v = nc.vector
    sc = nc.scalar
    gp = nc.gpsimd
    pe = nc.tensor

    _cms = [tc.tile_pool(name="main", bufs=1), tc.tile_pool(name="work", bufs=9),
            tc.tile_pool(name="psum", bufs=2, space="PSUM"),
            tc.tile_pool(name="psumd", bufs=2, space="PSUM")]
    pool, wk, psp, psd = [c.__enter__() for c in _cms]
    nc._dtw_pool_cms = _cms  # keep referenced; released at program end

    # ---------------- conv encoders ----------------
    def conv_chain(src_dram, cin0, chain):
        xp = pool.tile([128, 514], F32, tag=f"xpin{chain[0][0]}")
        nc.sync.dma_start(out=xp[0:cin0, 1:513], in_=din[src_dram][:])
        v.tensor_copy(out=xp[0:cin0, 0:1], in_=xp[0:cin0, 2:3])
        v.tensor_copy(out=xp[0:cin0, 513:514], in_=xp[0:cin0, 511:512])
        cur, ccur = xp, cin0
        for nm, ci, co in chain:
            wt = wk.tile([128, 3 * co], F32, tag="t512", name="wt")
            nc.sync.dma_start(out=wt[0:ci, :], in_=rap(din['w' + nm], 0, [[co, ci], [ci * co, 3], [1, co]]))
            bt = wk.tile([128, 1], F32, tag="tiny", name="bt")
            nc.sync.dma_start(out=bt[0:co, :], in_=din['b' + nm][:])
            ps = psd.tile([128, 512], F32, tag="big512")
            for dlt in range(3):
                pe.matmul(ps[0:co, :], wt[0:ci, dlt * co:(dlt + 1) * co],
                          cur[0:ccur, dlt:dlt + 512], start=(dlt == 0), stop=(dlt == 2))
            nxt = pool.tile([128, 514], F32, tag=f"xp{nm}")
            z = wk.tile([128, 512], F32, tag="t512", name="convz")
            v.tensor_scalar(out=z[0:co, :], in0=ps[0:co, :], scalar1=bt[0:co, :],
                            scalar2=None, op0=OP.add)
            z2 = wk.tile([128, 512], F32, tag="t512", name="convz2")
            v.tensor_scalar(out=z2[0:co, :], in0=z[0:co, :], scalar1=SLOPE,
                            scalar2=None, op0=OP.mult)
            v.tensor_tensor(out=nxt[0:co, 1:513], in0=z[0:co, :], in1=z2[0:co, :], op=OP.max)
            v.tensor_copy(out=nxt[0:co, 0:1], in_=nxt[0:co, 2:3])
            v.tensor_copy(out=nxt[0:co, 513:514], in_=nxt[0:co, 511:512])
            cur, ccur = nxt, co
        return cur  # [128, 514], latent in cols 1..513

    qlat = conv_chain('vec', 126, [('q1', 126, 126), ('q2', 126, 128), ('q3', 128, 128)])
    klat = conv_chain('music', 80, [('k1', 80, 80), ('k2', 80, 128), ('k3', 128, 128)])
    if DEBUG:
        nc.sync.dma_start(out=dbg['qlat'][:], in_=qlat[:, 1:513])
        nc.sync.dma_start(out=dbg['klat'][:], in_=klat[:, 1:513])

    # ---------------- dist matrix ----------------
    # |k|^2, |q|^2 via ones-matmul; G via (-2k)^T q; dist = sqrt(max(d2,0))
    ones_sb = pool.tile([128, 128], F32, tag="ones")
    v.memset(ones_sb[:], 1.0)
    ksq = wk.tile([128, 512], F32, tag="t512", name="ksq")
    v.tensor_tensor(out=ksq[:], in0=klat[:, 1:513], in1=klat[:, 1:513], op=OP.mult)
    qsq = wk.tile([128, 512], F32, tag="t512", name="qsq")
    v.tensor_tensor(out=qsq[:], in0=qlat[:, 1:513], in1=qlat[:, 1:513], op=OP.mult)
    psn = psd.tile([128, 512], F32, tag="big512")
    pe.matmul(psn[0:1, 0:512], ones_sb[:, 0:1], ksq[:], start=True, stop=True)
    psn2 = psd.tile([128, 512], F32, tag="big512")
    pe.matmul(psn2[0:1, 0:512], ones_sb[:, 0:1], qsq[:], start=True, stop=True)
    knq = pool.tile([128, 1024], F32, tag="knq")  # row0: cols 0:512=|k|^2, 512:1024=|q|^2
    v.tensor_copy(out=knq[0:1, 0:512], in_=psn[0:1, :])
    v.tensor_copy(out=knq[0:1, 512:1024], in_=psn2[0:1, :])
    ones1 = pool.tile([128, 512], F32, tag="ones1")
    v.memset(ones1[0:1, :], 1.0)
    m2k = wk.tile([128, 512], F32, tag="t512", name="m2k")
    v.tensor_scalar(out=m2k[:], in0=klat[:, 1:513], scalar1=-2.0, scalar2=None, op0=OP.mult)

    draw = pool.tile([128, 2048], F32, tag="draw")  # 4 chunks of [128,512] raw dist
    for t in range(4):
        psd2 = psd.tile([128, 512], F32, tag="big512")
        pe.matmul(psd2[:], m2k[:, t * 128:(t + 1) * 128], qlat[:, 1:513], start=True, stop=False)
        pe.matmul(psd2[:], knq[0:1, t * 128:(t + 1) * 128], ones1[0:1, 0:512], start=False, stop=False)
        pe.matmul(psd2[:], ones1[0:1, 0:128], knq[0:1, 512:1024], start=False, stop=True)
        dsq = wk.tile([128, 512], F32, tag="t512", name="dsq")
        v.tensor_scalar(out=dsq[:], in0=psd2[:], scalar1=0.0, scalar2=None, op0=OP.max)
        sc.activation(draw[:, t * 512:(t + 1) * 512], dsq[:], ACT.Sqrt)

    # min/max reduce
    red = wk.tile([128, 8], F32, tag="tiny", name="red")
    for t in range(4):
        v.tensor_reduce(out=red[:, t:t + 1], in_=draw[:, t * 512:(t + 1) * 512], axis=AX.X, op=OP.min)
        v.tensor_reduce(out=red[:, 4 + t:5 + t], in_=draw[:, t * 512:(t + 1) * 512], axis=AX.X, op=OP.max)
    red2 = wk.tile([128, 2], F32, tag="tiny", name="red2")
    v.tensor_reduce(out=red2[:, 0:1], in_=red[:, 0:4], axis=AX.X, op=OP.min)
    v.tensor_reduce(out=red2[:, 1:2], in_=red[:, 4:8], axis=AX.X, op=OP.max)
    # flatten partitions to free dim via DMA, then free reduce
    flat = pool.tile([128, 256], F32, tag="flat")
    nc.sync.dma_start(out=flat[0:1, 0:128], in_=red2[:, 0:1])
    nc.sync.dma_start(out=flat[0:1, 128:256], in_=red2[:, 1:2])
    mm = pool.tile([128, 2], F32, tag="mm")  # [1,2]: col0=-min col1=max
    v.tensor_reduce(out=mm[0:1, 0:1], in_=flat[0:1, 0:128], axis=AX.X, op=OP.min, negate=True)
    v.tensor_reduce(out=mm[0:1, 1:2], in_=flat[0:1, 128:256], axis=AX.X, op=OP.max)

    gmm = pool.tile([128, 2], F32, tag="gmm")
    if with_collective:
        _cccm = tc.tile_pool(name="ccdram", bufs=2, space="DRAM")
        nc._dtw_cc_cm = _cccm
        dramp = _cccm.__enter__()
        cc_in = dramp.tile([1, 2], F32)
        cc_out = dramp.tile([1, 2], F32)
        gp.dma_start(out=cc_in[:], in_=mm[0:1, 0:2])
        gp.collective_compute("AllReduce", OP.max,
                              replica_groups=[list(range(n_cores))],
                              ins=[cc_in.opt()], outs=[cc_out.opt()])
        gp.dma_start(out=gmm[0:1, 0:2], in_=cc_out[:])
    else:
        v.tensor_copy(out=gmm[0:1, 0:2], in_=mm[0:1, 0:2])

    # scale = 1/(max - min) = 1/(gmm[1] + gmm[0])  (gmm[0] = -min)
    sci = pool.tile([128, 2], F32, tag="sci")  # [1,1]: col0 = -min, col1 = scale
    v.tensor_copy(out=sci[0:1, 0:1], in_=gmm[0:1, 0:1])
    rngt = wk.tile([128, 1], F32, tag="tiny", name="rngt")
    v.tensor_tensor(out=rngt[0:1, :], in0=gmm[0:1, 1:2], in1=gmm[0:1, 0:1], op=OP.add)
    v.reciprocal(out=sci[0:1, 1:2], in_=rngt[0:1, :])
    # broadcast [1,2] -> [128,2] via ones matmul
    psb = psp.tile([128, 2], F32, tag="bc")
    pe.matmul(psb[:], ones_sb[0:1, :], sci[0:1, 0:2], start=True, stop=True)
    nmsc = pool.tile([128, 2], F32, tag="nmsc")
    v.tensor_copy(out=nmsc[:], in_=psb[:])

    # normalize and write dist out (+ padded staging copy for wavefront fills)
    zz = wk.tile([128, 32], F32, tag="t33", name="zz")
    v.memset(zz[:], 0.0)
    nc.sync.dma_start(out=rap(d_stage, 512 * 512, [[32, 128], [1, 32]]), in_=zz[:])
    for t in range(4):
        dn = wk.tile([128, 512], F32, tag="t512", name="dn")
        v.tensor_scalar(out=dn[:], in0=draw[:, t * 512:(t + 1) * 512],
                        scalar1=nmsc[:, 0:1], scalar2=nmsc[:, 1:2],
                        op0=OP.add, op1=OP.mult)
        nc.sync.dma_start(out=dist_out[t * 128:(t + 1) * 128, :], in_=dn[:])
        nc.sync.dma_start(out=rap(d_stage, t * 128 * 512, [[512, 128], [1, 512]]), in_=dn[:])

    # L - 1 broadcast (fp32)
    rl_sb = pool.tile([128, 2], F32, tag="rl")
    rli = wk.tile([128, 1], I32, tag="tinyi", name="rli")
    nc.sync.dma_start(out=rli[0:1, :], in_=din['rl'][:])
    v.tensor_copy(out=rl_sb[0:1, 0:1], in_=rli[0:1, :])   # int -> fp32 convert
    v.tensor_scalar(out=rl_sb[0:1, 1:2], in0=rl_sb[0:1, 0:1], scalar1=-1.0, scalar2=None, op0=OP.add)
    psb2 = psp.tile([128, 1], F32, tag="bc")
    pe.matmul(psb2[:], ones_sb[0:1, :], rl_sb[0:1, 1:2], start=True, stop=True)
    lbc = pool.tile([128, 1], F32, tag="lbc")
    v.tensor_copy(out=lbc[:], in_=psb2[:])

    # INF guards in cost_pad: row 0 (i=-1) data cols + guard col CO-1 all rows
    inf_t = pool.tile([128, 520], F32, tag="inf")
    v.memset(inf_t[:], LARGE)
    nc.sync.dma_start(out=rap(cost_pad, CO - 1, [[1, 514]]), in_=inf_t[0:1, 0:514])
    nc.sync.dma_start(out=rap(cost_pad, CS + CO - 1, [[CS, 513], [1, 1]]), in_=inf_t[0:1, 0:513])

    if STOP_AFTER == 'front':
        if getattr(nc, '_dtw_cc_cm', None) is not None:
            nc._dtw_cc_cm.__exit__(None, None, None)
        for c_ in reversed(nc._dtw_pool_cms):
            c_.__exit__(None, None, None)
        return
    inf11 = pool.tile([128, 1], F32, tag="inf11")
    v.memset(inf11[0:1, :], LARGE)
    eye_up = pool.tile([128, 128], F32, tag="eyeu")
    nc.sync.dma_start(out=eye_up[:], in_=din['eye_up'][:])
    eye_dn = pool.tile([128, 128], F32, tag="eyed")
    nc.sync.dma_start(out=eye_dn[:], in_=din['eye_dn'][:])

    # ---------------- forward wavefront ----------------
    CBg = [pool.tile([128, GW], F32, tag=f"cbg{g}", name=f"cbg{g}") for g in range(2)]
    # DWg slots are U=33 wide: col0 = 0.0 (static), data in [1:33].  The scan
    # runs 33 wide with data0 col0 = LARGE (static in utT) and data1 col0 = 0,
    # so out[0] = min(init, LARGE) + 0 = init — the guard column — fusing the
    # per-step guard copy into the scan.
    DWg = [pool.tile([128, ROUND * U], F32, tag=f"dwg{g}", name=f"dwg{g}") for g in range(2)]
    v.memset(DWg[0][:], 0.0)
    v.memset(DWg[1][:], 0.0)
    utT = pool.tile([128, U], F32, tag="utT")
    v.memset(utT[:], LARGE)
    v.memset(CBg[0][:], LARGE)
    v.memset(CBg[1][:], LARGE)
    # prime p0 row-0 cumsum start: initial of m=0 reads slot 28 (group1 slot 12) col 32 -> 0.0
    _pslot = (-4) % ROUND
    v.memset(CBg[1][0:1, _pslot * U + 32:_pslot * U + 33], 0.0)

    def cb_slice(m, c0, c1):
        g = (m // ROUND) % 2
        s0 = (m % ROUND) * U
        return CBg[g][:, s0 + c0:s0 + c1]

    def dw_slice(m):
        g = (m // ROUND) % 2
        s0 = (m % ROUND) * U
        return DWg[g][:, s0:s0 + U]

    def dfill(R):
        # dist[4p + t2][(4R + t1 - p)*W + f],  t = 4*t1 + t2; data to slot cols [1:33]
        g = R % 2
        for t2 in range(4):
            src = rap(d_stage, (ROUND // 4) * R * W + t2 * 512,
                      [[4 * 512 - W, 128], [W, ROUND // 4], [1, W]])
            dst = bass.AP(tensor=DWg[g][:].tensor, offset=DWg[g][:].offset + t2 * U + 1,
                          ap=[list(DWg[g][:].ap[0]), [4 * U, ROUND // 4], [1, W]])
            nc.sync.dma_start(out=dst, in_=src)

    def cost_deskew(R):
        g = R % 2
        for t2 in range(4):
            dst = rap(cost_pad, CS + CO + (ROUND // 4) * R * W + t2 * CS,
                      [[4 * CS - W, 128], [W, ROUND // 4], [1, W]])
            src = bass.AP(tensor=CBg[g][:].tensor, offset=CBg[g][:].offset + 1 + t2 * U,
                          ap=[list(CBg[g][:].ap[0]), [4 * U, ROUND // 4], [1, W]])
            nc.sync.dma_start(out=dst, in_=src)

    dfill(0)
    dfill(1)
    for m in range(NSTEP):
        if m % 4 == 0:
            ps = psp.tile([128, U], F32, tag="shift")
            pe.matmul(ps[:], eye_up[:], cb_slice(m - 1, 0, U), start=True, stop=True)
            scr = wk.tile([128, U], F32, tag="t33", name="scr")
            v.tensor_copy(out=scr[:], in_=ps[:])
            v.tensor_tensor(out=utT[:, 1:U], in0=scr[:, 0:W], in1=scr[:, 1:U], op=OP.min)
            if m <= 60:
                v.memset(utT[0:1, 1:U], LARGE)
        else:
            v.tensor_tensor(out=utT[:, 1:U], in0=cb_slice(m - 1, 0, W), in1=cb_slice(m - 1, 1, U), op=OP.min)
        init = cb_slice(m - 4, U - 1, U)
        v.tensor_tensor_scan(out=cb_slice(m, 0, U), data0=utT[:], data1=dw_slice(m),
                             initial=init, op0=OP.min, op1=OP.add)
        if m < 4:
            # guard col of the first 4 slots must stay LARGE (col -1 = INF),
            # not the scan-written init (partition 0 slot 0 init is 0.0)
            v.memset(cb_slice(m, 0, 1), LARGE)
        if m % ROUND == ROUND - 1:
            cost_deskew(m // ROUND)
            if m // ROUND + 2 < NROUND:
                dfill(m // ROUND + 2)
    cost_deskew(NROUND - 1)
    if STOP_AFTER == 'fwd':
        for c_ in reversed(nc._dtw_pool_cms):
            c_.__exit__(None, None, None)
        return

    # ---------------- bulk choice extraction ----------------
    iotaJ = pool.tile([128, 512], I32, tag="iJ")
    gp.iota(iotaJ[:], pattern=[[1, 512]], base=0, channel_multiplier=0)
    jf = pool.tile([128, 512], F32, tag="jf")
    v.tensor_copy(out=jf[:], in_=iotaJ[:])
    iotaI = pool.tile([128, 1], I32, tag="iI")
    gp.iota(iotaI[:], pattern=[[1, 1]], base=0, channel_multiplier=1)
    if_ = pool.tile([128, 1], F32, tag="if")
    v.tensor_copy(out=if_[:], in_=iotaI[:])

    for t in range(4):
        At = wk.tile([128, 513], F32, tag="t512", name="At")
        Bt = wk.tile([128, 513], F32, tag="t512", name="Bt")
        nc.sync.dma_start(out=At[:], in_=rap(cost_pad, (128 * t + 1) * CS + CO - 1, [[CS, 128], [1, 513]]))
        nc.sync.dma_start(out=Bt[:], in_=rap(cost_pad, (128 * t) * CS + CO - 1, [[CS, 128], [1, 513]]))
        m1 = wk.tile([128, 512], F32, tag="t512", name="m1")
        v.tensor_tensor(out=m1[:], in0=Bt[:, 0:512], in1=At[:, 0:512], op=OP.min)
        v.tensor_tensor(out=m1[:], in0=m1[:], in1=Bt[:, 1:513], op=OP.min)
        e0 = wk.tile([128, 512], F32, tag="t512", name="e0")
        v.tensor_tensor(out=e0[:], in0=Bt[:, 0:512], in1=m1[:], op=OP.is_equal)
        t1 = wk.tile([128, 512], F32, tag="t512", name="t1")
        v.tensor_tensor(out=t1[:], in0=At[:, 0:512], in1=m1[:], op=OP.is_equal)
        v.tensor_scalar(out=e0[:], in0=e0[:], scalar1=-1.0, scalar2=1.0, op0=OP.mult, op1=OP.add)
        v.tensor_scalar(out=t1[:], in0=t1[:], scalar1=-1.0, scalar2=2.0, op0=OP.mult, op1=OP.add)
        ct = wk.tile([128, 512], F32, tag="t512", name="ct")
        v.tensor_tensor(out=ct[:], in0=e0[:], in1=t1[:], op=OP.mult)
        nc.sync.dma_start(out=rap(c_pad, (128 * t + 1) * CPC, [[CPC, 128], [1, 512]]), in_=ct[:])

    pad3 = wk.tile([128, 520], F32, tag="t512", name="pad3")
    v.memset(pad3[:], 3.0)
    nc.sync.dma_start(out=rap(c_pad, 513 * CPC, [[1, 520]]), in_=pad3[0:1, 0:520])
    nc.sync.dma_start(out=rap(c_pad, 512, [[CPC, 515], [1, 1]]), in_=pad3[0:1, 0:515])

    for t in range(4):
        Cs = wk.tile([128, 513], F32, tag="t512", name="Cs")
        Cc = wk.tile([128, 513], F32, tag="t512", name="Cc")
        nc.sync.dma_start(out=Cs[:], in_=rap(c_pad, (128 * t + 2) * CPC, [[CPC, 128], [1, 513]]))
        nc.sync.dma_start(out=Cc[:], in_=rap(c_pad, (128 * t + 1) * CPC + 1, [[CPC, 128], [1, 513]]))
        e0s = wk.tile([128, 512], F32, tag="t512", name="e0s")
        v.tensor_scalar(out=e0s[:], in0=Cs[:, 1:513], scalar1=0.0, scalar2=None, op0=OP.is_equal)
        e2s = wk.tile([128, 512], F32, tag="t512", name="e2s")
        v.tensor_scalar(out=e2s[:], in0=Cs[:, 0:512], scalar1=2.0, scalar2=None, op0=OP.is_equal)
        c1s = wk.tile([128, 512], F32, tag="t512", name="c1s")
        v.tensor_scalar(out=c1s[:], in0=Cc[:, 0:512], scalar1=1.0, scalar2=None, op0=OP.is_equal)
        sI = wk.tile([128, 1], F32, tag="tiny", name="sI")
        v.tensor_scalar(out=sI[:], in0=if_[:], scalar1=float(128 * t), scalar2=None, op0=OP.add)
        v.tensor_tensor(out=sI[:], in0=sI[:], in1=lbc[:], op=OP.is_equal)
        sd = wk.tile([128, 512], F32, tag="t512", name="sd")
        v.tensor_scalar(out=sd[:], in0=jf[:], scalar1=lbc[:, 0:1], scalar2=None, op0=OP.is_equal)
        v.tensor_scalar(out=sd[:], in0=sd[:], scalar1=sI[:, 0:1], scalar2=None, op0=OP.mult)
        for tile_, padd in ((e0s, e0_pad), (e2s, e2_pad), (c1s, c1_pad), (sd, sd_pad)):
            nc.sync.dma_start(out=rap(padd, 128 * t * EC + CO_E, [[EC, 128], [1, 512]]), in_=tile_[:])

    # validity mask M[p, mb] = 1 iff 508 <= mb + 4p <= 571
    Ti = pool.tile([128, NSTEP + 4], I32, tag="Ti")
    gp.iota(Ti[:], pattern=[[1, NSTEP + 4]], base=0, channel_multiplier=4)
    Tf = pool.tile([128, NSTEP + 4], F32, tag="Tf")
    v.tensor_copy(out=Tf[:], in_=Ti[:])
    Ma = wk.tile([128, NSTEP + 4], F32, tag="Ma")
    v.tensor_scalar(out=Ma[:], in0=Tf[:], scalar1=507.5, scalar2=None, op0=OP.is_ge)
    Mv = pool.tile([128, NSTEP + 4], F32, tag="Mv")
    v.tensor_scalar(out=Mv[:], in0=Tf[:], scalar1=571.5, scalar2=None, op0=OP.is_le)
    v.tensor_tensor(out=Mv[:], in0=Mv[:], in1=Ma[:], op=OP.mult)

    if STOP_AFTER == 'bulk':
        for c_ in reversed(nc._dtw_pool_cms):
            c_.__exit__(None, None, None)
        return
    # ---------------- backward wavefront ----------------
    E0g = [pool.tile([128, ROUND * W], F32, tag=f"e0g{g}", name=f"e0g{g}") for g in range(2)]
    E2g = [pool.tile([128, ROUND * W], F32, tag=f"e2g{g}", name=f"e2g{g}") for g in range(2)]
    # C1g slots are U=33 wide: data [0:32), col32 = 1.0 (static) — reversed
    # scan's first element is (1.0*init) max 0.0 = init, writing the guard.
    C1g = [pool.tile([128, ROUND * U], F32, tag=f"c1g{g}", name=f"c1g{g}") for g in range(2)]
    SDg = [pool.tile([128, ROUND * W], F32, tag=f"sdg{g}", name=f"sdg{g}") for g in range(2)]
    Pg = [pool.tile([128, GW], F32, tag=f"pg{g}", name=f"pg{g}") for g in range(2)]
    v.memset(C1g[0][:], 1.0)
    v.memset(C1g[1][:], 1.0)
    e4x = pool.tile([128, U], F32, tag="e4x")
    v.memset(e4x[:], 0.0)
    e6s = pool.tile([128, W], F32, tag="e6s")
    praw = pool.tile([128, U], F32, tag="praw")
    v.memset(Pg[0][:], 0.0)
    v.memset(Pg[1][:], 0.0)

    def p_slice(mb, c0, c1):
        g = (mb // ROUND) % 2
        s0 = (mb % ROUND) * U
        return Pg[g][:, s0 + c0:s0 + c1]

    def ew_slice(Wg, mb):
        g = (mb // ROUND) % 2
        s0 = (mb % ROUND) * W
        return Wg[g][:, s0:s0 + W]

    def c1_slice(mb):
        g = (mb // ROUND) % 2
        s0 = (mb % ROUND) * U
        return C1g[g][:, s0:s0 + U]

    def bfill(R, padd, Wg, sw=W):
        # addr = p*(4EC - W) + (3-b)*EC + (142-4R-a)*W + f + CO_E,  t = 4a + b
        g = R % 2
        for b in range(4):
            src = rap(padd, (3 - b) * EC + (142 - (ROUND // 4) * R) * W + CO_E,
                      [[4 * EC - W, 128], [-W, ROUND // 4], [1, W]])
            dst = bass.AP(tensor=Wg[g][:].tensor, offset=Wg[g][:].offset + b * sw,
                          ap=[list(Wg[g][:].ap[0]), [4 * sw, ROUND // 4], [1, W]])
            nc.sync.dma_start(out=dst, in_=src)

    def p_deskew(R):
        g = R % 2
        for b in range(4):
            dst = rap(p_pad, (3 - b) * EC + (142 - (ROUND // 4) * R) * W + CO_E,
                      [[4 * EC - W, 128], [-W, ROUND // 4], [1, W]])
            src = bass.AP(tensor=Pg[g][:].tensor, offset=Pg[g][:].offset + b * U,
                          ap=[list(Pg[g][:].ap[0]), [4 * U, ROUND // 4], [1, W]])
            nc.sync.dma_start(out=dst, in_=src)

    SD_LAST_ROUND = 300 // ROUND  # Sd only read at mb <= 300 (L >= 256)
    for padd, Wg, sw in ((e0_pad, E0g, W), (e2_pad, E2g, W), (c1_pad, C1g, U), (sd_pad, SDg, W)):
        bfill(0, padd, Wg, sw)
        bfill(1, padd, Wg, sw)
    for mb in range(NSTEP):
        if mb % 4 == 0:
            ps2 = psp.tile([128, U], F32, tag="shift")
            pe.matmul(ps2[:], eye_dn[:], p_slice(mb - 1, 0, U), start=True, stop=True)
            v.tensor_tensor(out=e4x[:, 0:W], in0=ew_slice(E0g, mb), in1=ps2[:, 1:U], op=OP.mult)
            v.tensor_tensor(out=e6s[:], in0=ew_slice(E2g, mb), in1=ps2[:, 0:W], op=OP.mult)
        else:
            v.tensor_tensor(out=e4x[:, 0:W], in0=ew_slice(E0g, mb), in1=p_slice(mb - 1, 1, U), op=OP.mult)
            v.tensor_tensor(out=e6s[:], in0=ew_slice(E2g, mb), in1=p_slice(mb - 1, 0, W), op=OP.mult)
        v.tensor_tensor(out=e4x[:, 0:W], in0=e4x[:, 0:W], in1=e6s[:], op=OP.max)
        if mb <= 300:
            v.tensor_tensor(out=e4x[:, 0:W], in0=e4x[:, 0:W], in1=ew_slice(SDg, mb), op=OP.max)
        init = p_slice(mb - 4, 0, 1)
        v.tensor_tensor_scan(out=praw[:, ::-1], data0=c1_slice(mb)[:, ::-1],
                             data1=e4x[:, ::-1], initial=init, op0=OP.mult, op1=OP.max)
        v.tensor_scalar(out=p_slice(mb, 0, U), in0=praw[:],
                        scalar1=Mv[:, mb:mb + 1], scalar2=0.5, op0=OP.mult, op1=OP.is_ge)
        if mb % ROUND == ROUND - 1:
            p_deskew(mb // ROUND)
            if mb // ROUND + 2 < NROUND:
                for padd, Wg, sw in ((e0_pad, E0g, W), (e2_pad, E2g, W), (c1_pad, C1g, U)):
                    bfill(mb // ROUND + 2, padd, Wg, sw)
                if mb // ROUND + 2 <= SD_LAST_ROUND:
                    bfill(mb // ROUND + 2, sd_pad, SDg)
    p_deskew(NROUND - 1)

    # ---------------- path01 repack ----------------
    for t in range(4):
        pt = wk.tile([128, 512], F32, tag="t512", name="pt")
        nc.sync.dma_start(out=pt[:], in_=rap(p_pad, 128 * t * EC + CO_E, [[EC, 128], [1, 512]]))
        nc.sync.dma_start(out=path_out[128 * t:128 * (t + 1), :], in_=pt[:])
    if DEBUG:
        for t in range(4):
            ctd = wk.tile([128, 512], F32, tag="t512", name="ctd")
            nc.sync.dma_start(out=ctd[:], in_=rap(cost_pad, (128 * t + 1) * CS + CO, [[CS, 128], [1, 512]]))
            nc.sync.dma_start(out=dbg['cost'][128 * t:128 * (t + 1), :], in_=ctd[:])
            ccd = wk.tile([128, 512], F32, tag="t512", name="ccd")
            nc.sync.dma_start(out=ccd[:], in_=rap(c_pad, (128 * t + 1) * CPC, [[CPC, 128], [1, 512]]))
            nc.sync.dma_start(out=dbg['C'][128 * t:128 * (t + 1), :], in_=ccd[:])
    if getattr(nc, '_dtw_cc_cm', None) is not None:
        nc._dtw_cc_cm.__exit__(None, None, None)
    for c_ in reversed(nc._dtw_pool_cms):
        c_.__exit__(None, None, None)


def make_host_inputs(vec_b, music_b, rl_b, weights):
    """Per-core in_map dict from one sample's data. weights: dict of full arrays."""
    m = {
        'vec': np.ascontiguousarray(vec_b, np.float32),
        'music': np.ascontiguousarray(music_b, np.float32),
        'rl': np.asarray([rl_b], np.int32),
        'eye_up': np.eye(128, k=1).astype(np.float32),
        'eye_dn': np.eye(128, k=-1).astype(np.float32),
        'ones_bc': np.ones((1, 128), np.float32),
    }
    for nm in ('q1', 'q2', 'q3', 'k1', 'k2', 'k3'):
        w = weights['w' + nm]  # [Cout, Cin, 3]
        m['w' + nm] = np.ascontiguousarray(w.transpose(2, 1, 0), np.float32)  # [3, Cin, Cout]
        m['b' + nm] = np.ascontiguousarray(weights['b' + nm].reshape(-1, 1), np.float32)
    return m


# ---------------------------------------------------------------- host entry
_CACHED = {}


def _get_nc():
    if 'nc' not in _CACHED:
        _CACHED['nc'] = build_program(n_cores=8, with_collective=True)
    return _CACHED['nc']


def kernel(vec, music, real_length, qw1, qb1, qw2, qb2, qw3, qb3,
           kw1, kb1, kw2, kb2, kw3, kb3):
    from concourse.bass_utils import run_bass_kernel_spmd
    weights = {'wq1': qw1, 'bq1': qb1, 'wq2': qw2, 'bq2': qb2,
               'wq3': qw3, 'bq3': qb3, 'wk1': kw1, 'bk1': kb1,
               'wk2': kw2, 'bk2': kb2, 'wk3': kw3, 'bk3': kb3}
    weights = {k: np.asarray(v_, np.float32) for k, v_ in weights.items()}
    vec = np.asarray(vec, np.float32)
    music = np.asarray(music, np.float32)
    rl = np.asarray(real_length, np.int32)
    nc = _get_nc()
    in_maps = [make_host_inputs(vec[b], music[b], rl[b], weights) for b in range(8)]
    res = run_bass_kernel_spmd(nc, in_maps, list(range(8)))
    path01 = np.stack([res.results[c]['path01'] for c in range(8)]).astype(np.float32)
    dist = np.stack([res.results[c]['dist'] for c in range(8)]).astype(np.float32)
    return (path01, dist)



# revision 9
# speedup vs baseline: 1.0924x; 1.0924x over previous
"""Bass/Tile program for nn_DTWModel on TRN2: conv encoders + euclidean dist
+ global min-max norm + exact DTW (forward wavefront row-scans, bulk choice
extraction, backward path-marking wavefront).

Layout summary (per core, one sample):
- rows r=0..511 of the DTW matrix; partition p owns rows 4p..4p+3.
- forward: unit (k,s) = (row 4p+k, col-strip s of width W=32) processed at
  step m = 4p + 4s + k.  All partitions share step-uniform APs via a
  32-slot rotating window (2 group tiles of 16 slots); slot = m % 32.
- CB slot layout: [guard][32 cost values]; guard(slot m) = last value of
  slot m-4 (same row, previous strip) = cost[r][s*W-1].
- scan: state = (u min state) + d  == min(min(pd,up),left)+d of reference.
- u = min(CB(m-1)[0:32], CB(m-1)[1:33]) = min(pd, up) from row r-1.
- k=0 rows need row 4p-1 from partition p-1: PE matmul with shifted
  identity moves the slot down one partition (psum[p] = slot[p-1]).
- cost deskewed to DRAM via p-linear strided DMAs every 16 steps.
- bulk phase recomputes choices C from cost with reference tie-break, then
  static masks E0s/E2s/c1s and seed Sd, all written to padded DRAM.
- backward: P[i][j] = max(Sd, E0s*P[i+1][j+1], E2s*P[i+1][j], c1s-scan)
  processed as mirrored wavefront with reversed ttscan; P masked NaN-proof
  by validity mask M via (P*M) is_ge 0.5.
"""
import sys as _sys
if '/opt/trn_rl_repo' not in _sys.path:
    _sys.path.insert(0, '/opt/trn_rl_repo')
import numpy as np
import concourse.bass as bass
import concourse.mybir as mybir
from concourse.vector_clock import ScopedClock
from concourse.tile import TileContext

F32 = mybir.dt.float32
I32 = mybir.dt.int32
OP = mybir.AluOpType
ACT = mybir.ActivationFunctionType
AX = mybir.AxisListType

LARGE = float(np.float32(1e30))
SLOPE = float(np.float32(0.2))
DEBUG = False
STOP_AFTER = None  # 'front'|'fwd'|'bulk'|None

W = 32          # strip width
U = 33          # slot width (guard + W)
S = 512 // W    # strips per row = 16
NSTEP = 4 * 127 + 4 * (S - 1) + 3 + 1   # 572 steps, m in [0, 572)
ROUND = 32
NROUND = (NSTEP + ROUND - 1) // ROUND
GW = ROUND * U  # group tile width = 528

# cost_pad DRAM layout
CS = 4672       # row stride (cols)
CO = 4064       # data col offset; col CO-1 = INF guard (j=-1)
CROWS = 514     # row i stored at row i+1; row 0 = INF

# C_pad layout: row r stored at r+1; rows 0 unused, row 513 = 3.0 (virtual r=512)
CPR, CPC = 515, 520

# E/Sd/P pads
EC = 8672
CO_E = 4096
EROWS = 512


class SplitDrainTileContext(TileContext):
    """Final drain must carry <=1 sem wait for this neuronxcc."""

    def _drain_and_barrier(self, tick_clock, wait_clock):
        drain_inst = self.nc.sync.drain()
        wait_clock.add_sem_waits(
            drain_inst.ins, ScopedClock({None: tick_clock.global_clock})
        )
        si = drain_inst.ins.sync_info
        waits = list(si.on_wait or [])
        if len(waits) > 1:
            si.on_wait[:] = waits[:1]
            for w_ in waits[1:]:
                nop = self.nc.sync.nop(nofuse=True, hint="split_drain_wait")
                nsi = nop.ins.sync_info
                if nsi is None:
                    nop.ins.sync_info = mybir.SyncInfo(on_wait=[w_], on_update=[])
                else:
                    nsi.on_wait.append(w_)
        self.nc.all_engine_barrier()
        assert self.sems is not None
        popped = self.nc._tile_sem_poison_stack.pop()
        assert popped is self._sem_poison
        self.nc.clear_and_free_semaphores(list(self.sems.allocated().values()))
        self.nc.all_engine_barrier()


def rap(t, offset, ap):
    return bass.AP(tensor=t[:].tensor, offset=int(offset), ap=[[int(a), int(b)] for a, b in ap])


def build_program(n_cores=8, with_collective=True):
    nc = bass.Bass("TRN2", target_bir_lowering=False, debug=False,
                   num_devices=n_cores)

    # ---------------- dram tensors ----------------
    din = {}
    din['vec'] = nc.dram_tensor("vec", [126, 512], F32, kind="ExternalInput")
    din['music'] = nc.dram_tensor("music", [80, 512], F32, kind="ExternalInput")
    din['rl'] = nc.dram_tensor("rl", [1], I32, kind="ExternalInput")
    wspec = [('q1', 126, 126), ('q2', 126, 128), ('q3', 128, 128),
             ('k1', 80, 80), ('k2', 80, 128), ('k3', 128, 128)]
    for nm, ci, co in wspec:
        din['w' + nm] = nc.dram_tensor("w" + nm, [3, ci, co], F32, kind="ExternalInput")
        din['b' + nm] = nc.dram_tensor("b" + nm, [co, 1], F32, kind="ExternalInput")
    din['eye_up'] = nc.dram_tensor("eye_up", [128, 128], F32, kind="ExternalInput")
    din['eye_dn'] = nc.dram_tensor("eye_dn", [128, 128], F32, kind="ExternalInput")
    din['ones_bc'] = nc.dram_tensor("ones_bc", [1, 128], F32, kind="ExternalInput")

    dist_out = nc.dram_tensor("dist", [512, 512], F32, kind="ExternalOutput")
    path_out = nc.dram_tensor("path01", [512, 512], F32, kind="ExternalOutput")

    cost_pad = nc.dram_tensor("cost_pad", [CROWS * CS], F32)
    c_pad = nc.dram_tensor("c_pad", [CPR * CPC], F32)
    e0_pad = nc.dram_tensor("e0_pad", [EROWS * EC], F32)
    e2_pad = nc.dram_tensor("e2_pad", [EROWS * EC], F32)
    c1_pad = nc.dram_tensor("c1_pad", [EROWS * EC], F32)
    sd_pad = nc.dram_tensor("sd_pad", [EROWS * EC], F32)
    p_pad = nc.dram_tensor("p_pad", [EROWS * EC], F32)
    d_stage = nc.dram_tensor("d_stage", [524 * 512], F32)

    dbg = {}
    if DEBUG:
        dbg['qlat'] = nc.dram_tensor("dbg_qlat", [128, 512], F32, kind="ExternalOutput")
        dbg['klat'] = nc.dram_tensor("dbg_klat", [128, 512], F32, kind="ExternalOutput")
        dbg['cost'] = nc.dram_tensor("dbg_cost", [512, 512], F32, kind="ExternalOutput")
        dbg['C'] = nc.dram_tensor("dbg_C", [512, 512], F32, kind="ExternalOutput")

    with SplitDrainTileContext(nc) as tc:
        _build_body(nc, tc, din, dist_out, path_out, cost_pad, c_pad,
                    e0_pad, e2_pad, c1_pad, sd_pad, p_pad, d_stage,
                    with_collective, n_cores, dbg)
    _split_multi_waits(nc)
    return nc


def _split_multi_waits(nc, max_waits=1):
    """This neuronxcc rejects instructions with more than ~1-2 sync waits.
    Move extra waits onto same-engine NoOps inserted just before."""
    import bass_rust as _br
    ctr = [0]
    for f in nc.m.functions:
        for bb in f.blocks:
            newlist = []
            for inst in bb.instructions:
                si = inst.sync_info
                waits = list(si.on_wait) if (si and si.on_wait) else []
                if len(waits) > max_waits:
                    keep = waits[:max_waits]
                    extra = waits[max_waits:]
                    si.on_wait[:] = keep
                    for w_ in extra:
                        ctr[0] += 1
                        nop = _br.InstNoOp(name=f"waitsplit_{ctr[0]}")
                        nop.engine = inst.engine
                        nop.sync_info = mybir.SyncInfo(on_wait=[w_], on_update=[])
                        nc.register_instruction(nop, overwrite=True)
                        newlist.append(nop)
                newlist.append(inst)
            if ctr[0]:
                bb.instructions[:] = newlist
    return ctr[0]


def _build_body(nc, tc, din, dist_out, path_out, cost_pad, c_pad,
                e0_pad, e2_pad, c1_pad, sd_pad, p_pad, d_stage, with_collective,
                n_cores, dbg):
    v = nc.vector
    sc = nc.scalar
    gp = nc.gpsimd
    pe = nc.tensor

    _cms = [tc.tile_pool(name="main", bufs=1), tc.tile_pool(name="work", bufs=9),
            tc.tile_pool(name="psum", bufs=2, space="PSUM"),
            tc.tile_pool(name="psumd", bufs=2, space="PSUM")]
    pool, wk, psp, psd = [c.__enter__() for c in _cms]
    nc._dtw_pool_cms = _cms  # keep referenced; released at program end

    # ---------------- conv encoders ----------------
    def conv_chain(src_dram, cin0, chain):
        xp = pool.tile([128, 514], F32, tag=f"xpin{chain[0][0]}")
        nc.sync.dma_start(out=xp[0:cin0, 1:513], in_=din[src_dram][:])
        v.tensor_copy(out=xp[0:cin0, 0:1], in_=xp[0:cin0, 2:3])
        v.tensor_copy(out=xp[0:cin0, 513:514], in_=xp[0:cin0, 511:512])
        cur, ccur = xp, cin0
        for nm, ci, co in chain:
            wt = wk.tile([128, 3 * co], F32, tag="t512", name="wt")
            nc.sync.dma_start(out=wt[0:ci, :], in_=rap(din['w' + nm], 0, [[co, ci], [ci * co, 3], [1, co]]))
            bt = wk.tile([128, 1], F32, tag="tiny", name="bt")
            nc.sync.dma_start(out=bt[0:co, :], in_=din['b' + nm][:])
            ps = psd.tile([128, 512], F32, tag="big512")
            for dlt in range(3):
                pe.matmul(ps[0:co, :], wt[0:ci, dlt * co:(dlt + 1) * co],
                          cur[0:ccur, dlt:dlt + 512], start=(dlt == 0), stop=(dlt == 2))
            nxt = pool.tile([128, 514], F32, tag=f"xp{nm}")
            z = wk.tile([128, 512], F32, tag="t512", name="convz")
            v.tensor_scalar(out=z[0:co, :], in0=ps[0:co, :], scalar1=bt[0:co, :],
                            scalar2=None, op0=OP.add)
            z2 = wk.tile([128, 512], F32, tag="t512", name="convz2")
            v.tensor_scalar(out=z2[0:co, :], in0=z[0:co, :], scalar1=SLOPE,
                            scalar2=None, op0=OP.mult)
            v.tensor_tensor(out=nxt[0:co, 1:513], in0=z[0:co, :], in1=z2[0:co, :], op=OP.max)
            v.tensor_copy(out=nxt[0:co, 0:1], in_=nxt[0:co, 2:3])
            v.tensor_copy(out=nxt[0:co, 513:514], in_=nxt[0:co, 511:512])
            cur, ccur = nxt, co
        return cur  # [128, 514], latent in cols 1..513

    qlat = conv_chain('vec', 126, [('q1', 126, 126), ('q2', 126, 128), ('q3', 128, 128)])
    klat = conv_chain('music', 80, [('k1', 80, 80), ('k2', 80, 128), ('k3', 128, 128)])
    if DEBUG:
        nc.sync.dma_start(out=dbg['qlat'][:], in_=qlat[:, 1:513])
        nc.sync.dma_start(out=dbg['klat'][:], in_=klat[:, 1:513])

    # ---------------- dist matrix ----------------
    # |k|^2, |q|^2 via ones-matmul; G via (-2k)^T q; dist = sqrt(max(d2,0))
    ones_sb = pool.tile([128, 128], F32, tag="ones")
    v.memset(ones_sb[:], 1.0)
    ksq = wk.tile([128, 512], F32, tag="t512", name="ksq")
    v.tensor_tensor(out=ksq[:], in0=klat[:, 1:513], in1=klat[:, 1:513], op=OP.mult)
    qsq = wk.tile([128, 512], F32, tag="t512", name="qsq")
    v.tensor_tensor(out=qsq[:], in0=qlat[:, 1:513], in1=qlat[:, 1:513], op=OP.mult)
    psn = psd.tile([128, 512], F32, tag="big512")
    pe.matmul(psn[0:1, 0:512], ones_sb[:, 0:1], ksq[:], start=True, stop=True)
    psn2 = psd.tile([128, 512], F32, tag="big512")
    pe.matmul(psn2[0:1, 0:512], ones_sb[:, 0:1], qsq[:], start=True, stop=True)
    knq = pool.tile([128, 1024], F32, tag="knq")  # row0: cols 0:512=|k|^2, 512:1024=|q|^2
    v.tensor_copy(out=knq[0:1, 0:512], in_=psn[0:1, :])
    v.tensor_copy(out=knq[0:1, 512:1024], in_=psn2[0:1, :])
    ones1 = pool.tile([128, 512], F32, tag="ones1")
    v.memset(ones1[0:1, :], 1.0)
    m2k = wk.tile([128, 512], F32, tag="t512", name="m2k")
    v.tensor_scalar(out=m2k[:], in0=klat[:, 1:513], scalar1=-2.0, scalar2=None, op0=OP.mult)

    draw = pool.tile([128, 2048], F32, tag="draw")  # 4 chunks of [128,512] raw dist
    for t in range(4):
        psd2 = psd.tile([128, 512], F32, tag="big512")
        pe.matmul(psd2[:], m2k[:, t * 128:(t + 1) * 128], qlat[:, 1:513], start=True, stop=False)
        pe.matmul(psd2[:], knq[0:1, t * 128:(t + 1) * 128], ones1[0:1, 0:512], start=False, stop=False)
        pe.matmul(psd2[:], ones1[0:1, 0:128], knq[0:1, 512:1024], start=False, stop=True)
        dsq = wk.tile([128, 512], F32, tag="t512", name="dsq")
        v.tensor_scalar(out=dsq[:], in0=psd2[:], scalar1=0.0, scalar2=None, op0=OP.max)
        sc.activation(draw[:, t * 512:(t + 1) * 512], dsq[:], ACT.Sqrt)

    # min/max reduce
    red = wk.tile([128, 8], F32, tag="tiny", name="red")
    for t in range(4):
        v.tensor_reduce(out=red[:, t:t + 1], in_=draw[:, t * 512:(t + 1) * 512], axis=AX.X, op=OP.min)
        v.tensor_reduce(out=red[:, 4 + t:5 + t], in_=draw[:, t * 512:(t + 1) * 512], axis=AX.X, op=OP.max)
    red2 = wk.tile([128, 2], F32, tag="tiny", name="red2")
    v.tensor_reduce(out=red2[:, 0:1], in_=red[:, 0:4], axis=AX.X, op=OP.min)
    v.tensor_reduce(out=red2[:, 1:2], in_=red[:, 4:8], axis=AX.X, op=OP.max)
    # flatten partitions to free dim via DMA, then free reduce
    flat = pool.tile([128, 256], F32, tag="flat")
    nc.sync.dma_start(out=flat[0:1, 0:128], in_=red2[:, 0:1])
    nc.sync.dma_start(out=flat[0:1, 128:256], in_=red2[:, 1:2])
    mm = pool.tile([128, 2], F32, tag="mm")  # [1,2]: col0=-min col1=max
    v.tensor_reduce(out=mm[0:1, 0:1], in_=flat[0:1, 0:128], axis=AX.X, op=OP.min, negate=True)
    v.tensor_reduce(out=mm[0:1, 1:2], in_=flat[0:1, 128:256], axis=AX.X, op=OP.max)

    gmm = pool.tile([128, 2], F32, tag="gmm")
    if with_collective:
        _cccm = tc.tile_pool(name="ccdram", bufs=2, space="DRAM")
        nc._dtw_cc_cm = _cccm
        dramp = _cccm.__enter__()
        cc_in = dramp.tile([1, 2], F32)
        cc_out = dramp.tile([1, 2], F32)
        gp.dma_start(out=cc_in[:], in_=mm[0:1, 0:2])
        gp.collective_compute("AllReduce", OP.max,
                              replica_groups=[list(range(n_cores))],
                              ins=[cc_in.opt()], outs=[cc_out.opt()])
        gp.dma_start(out=gmm[0:1, 0:2], in_=cc_out[:])
    else:
        v.tensor_copy(out=gmm[0:1, 0:2], in_=mm[0:1, 0:2])

    # scale = 1/(max - min) = 1/(gmm[1] + gmm[0])  (gmm[0] = -min)
    sci = pool.tile([128, 2], F32, tag="sci")  # [1,1]: col0 = -min, col1 = scale
    v.tensor_copy(out=sci[0:1, 0:1], in_=gmm[0:1, 0:1])
    rngt = wk.tile([128, 1], F32, tag="tiny", name="rngt")
    v.tensor_tensor(out=rngt[0:1, :], in0=gmm[0:1, 1:2], in1=gmm[0:1, 0:1], op=OP.add)
    v.reciprocal(out=sci[0:1, 1:2], in_=rngt[0:1, :])
    # broadcast [1,2] -> [128,2] via ones matmul
    psb = psp.tile([128, 2], F32, tag="bc")
    pe.matmul(psb[:], ones_sb[0:1, :], sci[0:1, 0:2], start=True, stop=True)
    nmsc = pool.tile([128, 2], F32, tag="nmsc")
    v.tensor_copy(out=nmsc[:], in_=psb[:])

    # normalize and write dist out (+ padded staging copy for wavefront fills)
    zz = wk.tile([128, 32], F32, tag="t33", name="zz")
    v.memset(zz[:], 0.0)
    nc.sync.dma_start(out=rap(d_stage, 512 * 512, [[32, 128], [1, 32]]), in_=zz[:])
    for t in range(4):
        dn = wk.tile([128, 512], F32, tag="t512", name="dn")
        v.tensor_scalar(out=dn[:], in0=draw[:, t * 512:(t + 1) * 512],
                        scalar1=nmsc[:, 0:1], scalar2=nmsc[:, 1:2],
                        op0=OP.add, op1=OP.mult)
        nc.sync.dma_start(out=dist_out[t * 128:(t + 1) * 128, :], in_=dn[:])
        nc.sync.dma_start(out=rap(d_stage, t * 128 * 512, [[512, 128], [1, 512]]), in_=dn[:])

    # L - 1 broadcast (fp32)
    rl_sb = pool.tile([128, 2], F32, tag="rl")
    rli = wk.tile([128, 1], I32, tag="tinyi", name="rli")
    nc.sync.dma_start(out=rli[0:1, :], in_=din['rl'][:])
    v.tensor_copy(out=rl_sb[0:1, 0:1], in_=rli[0:1, :])   # int -> fp32 convert
    v.tensor_scalar(out=rl_sb[0:1, 1:2], in0=rl_sb[0:1, 0:1], scalar1=-1.0, scalar2=None, op0=OP.add)
    psb2 = psp.tile([128, 1], F32, tag="bc")
    pe.matmul(psb2[:], ones_sb[0:1, :], rl_sb[0:1, 1:2], start=True, stop=True)
    lbc = pool.tile([128, 1], F32, tag="lbc")
    v.tensor_copy(out=lbc[:], in_=psb2[:])

    # INF guards in cost_pad: row 0 (i=-1) data cols + guard col CO-1 all rows
    inf_t = pool.tile([128, 520], F32, tag="inf")
    v.memset(inf_t[:], LARGE)
    nc.sync.dma_start(out=rap(cost_pad, CO - 1, [[1, 514]]), in_=inf_t[0:1, 0:514])
    nc.sync.dma_start(out=rap(cost_pad, CS + CO - 1, [[CS, 513], [1, 1]]), in_=inf_t[0:1, 0:513])

    if STOP_AFTER == 'front':
        if getattr(nc, '_dtw_cc_cm', None) is not None:
            nc._dtw_cc_cm.__exit__(None, None, None)
        for c_ in reversed(nc._dtw_pool_cms):
            c_.__exit__(None, None, None)
        return
    inf11 = pool.tile([128, 1], F32, tag="inf11")
    v.memset(inf11[0:1, :], LARGE)
    eye_up = pool.tile([128, 128], F32, tag="eyeu")
    nc.sync.dma_start(out=eye_up[:], in_=din['eye_up'][:])
    eye_dn = pool.tile([128, 128], F32, tag="eyed")
    nc.sync.dma_start(out=eye_dn[:], in_=din['eye_dn'][:])

    # ---------------- forward wavefront ----------------
    CBg = [pool.tile([128, GW], F32, tag=f"cbg{g}", name=f"cbg{g}") for g in range(2)]
    # DWg slots are U=33 wide: col0 = 0.0 (static), data in [1:33].  The scan
    # runs 33 wide with data0 col0 = LARGE (static in utT) and data1 col0 = 0,
    # so out[0] = min(init, LARGE) + 0 = init — the guard column — fusing the
    # per-step guard copy into the scan.
    DWg = [pool.tile([128, ROUND * U], F32, tag=f"dwg{g}", name=f"dwg{g}") for g in range(2)]
    v.memset(DWg[0][:], 0.0)
    v.memset(DWg[1][:], 0.0)
    utT = pool.tile([128, U], F32, tag="utT")
    v.memset(utT[:], LARGE)
    v.memset(CBg[0][:], LARGE)
    v.memset(CBg[1][:], LARGE)
    # prime p0 row-0 cumsum start: initial of m=0 reads slot 28 (group1 slot 12) col 32 -> 0.0
    _pslot = (-4) % ROUND
    v.memset(CBg[1][0:1, _pslot * U + 32:_pslot * U + 33], 0.0)

    def cb_slice(m, c0, c1):
        g = (m // ROUND) % 2
        s0 = (m % ROUND) * U
        return CBg[g][:, s0 + c0:s0 + c1]

    def dw_slice(m):
        g = (m // ROUND) % 2
        s0 = (m % ROUND) * U
        return DWg[g][:, s0:s0 + U]

    def dfill(R):
        # dist[4p + t2][(4R + t1 - p)*W + f],  t = 4*t1 + t2; data to slot cols [1:33]
        g = R % 2
        for t2 in range(4):
            src = rap(d_stage, (ROUND // 4) * R * W + t2 * 512,
                      [[4 * 512 - W, 128], [W, ROUND // 4], [1, W]])
            dst = bass.AP(tensor=DWg[g][:].tensor, offset=DWg[g][:].offset + t2 * U + 1,
                          ap=[list(DWg[g][:].ap[0]), [4 * U, ROUND // 4], [1, W]])
            nc.sync.dma_start(out=dst, in_=src)

    def cost_deskew(R):
        g = R % 2
        for t2 in range(4):
            dst = rap(cost_pad, CS + CO + (ROUND // 4) * R * W + t2 * CS,
                      [[4 * CS - W, 128], [W, ROUND // 4], [1, W]])
            src = bass.AP(tensor=CBg[g][:].tensor, offset=CBg[g][:].offset + 1 + t2 * U,
                          ap=[list(CBg[g][:].ap[0]), [4 * U, ROUND // 4], [1, W]])
            nc.sync.dma_start(out=dst, in_=src)

    dfill(0)
    dfill(1)
    for m in range(NSTEP):
        if m % 4 == 0:
            ps = psp.tile([128, U], F32, tag="shift")
            pe.matmul(ps[:], eye_up[:], cb_slice(m - 1, 0, U), start=True, stop=True)
            scr = wk.tile([128, U], F32, tag="t33", name="scr")
            v.tensor_copy(out=scr[:], in_=ps[:])
            v.tensor_tensor(out=utT[:, 1:U], in0=scr[:, 0:W], in1=scr[:, 1:U], op=OP.min)
            if m <= 60:
                v.memset(utT[0:1, 1:U], LARGE)
        else:
            v.tensor_tensor(out=utT[:, 1:U], in0=cb_slice(m - 1, 0, W), in1=cb_slice(m - 1, 1, U), op=OP.min)
        init = cb_slice(m - 4, U - 1, U)
        v.tensor_tensor_scan(out=cb_slice(m, 0, U), data0=utT[:], data1=dw_slice(m),
                             initial=init, op0=OP.min, op1=OP.add)
        if m < 4:
            # guard col of the first 4 slots must stay LARGE (col -1 = INF),
            # not the scan-written init (partition 0 slot 0 init is 0.0)
            v.memset(cb_slice(m, 0, 1), LARGE)
        if m % ROUND == ROUND - 1:
            cost_deskew(m // ROUND)
            if m // ROUND + 2 < NROUND:
                dfill(m // ROUND + 2)
    cost_deskew(NROUND - 1)
    if STOP_AFTER == 'fwd':
        for c_ in reversed(nc._dtw_pool_cms):
            c_.__exit__(None, None, None)
        return

    # ---------------- bulk choice extraction ----------------
    iotaJ = pool.tile([128, 512], I32, tag="iJ")
    gp.iota(iotaJ[:], pattern=[[1, 512]], base=0, channel_multiplier=0)
    jf = pool.tile([128, 512], F32, tag="jf")
    v.tensor_copy(out=jf[:], in_=iotaJ[:])
    iotaI = pool.tile([128, 1], I32, tag="iI")
    gp.iota(iotaI[:], pattern=[[1, 1]], base=0, channel_multiplier=1)
    if_ = pool.tile([128, 1], F32, tag="if")
    v.tensor_copy(out=if_[:], in_=iotaI[:])

    for t in range(4):
        At = wk.tile([128, 513], F32, tag="t512", name="At")
        Bt = wk.tile([128, 513], F32, tag="t512", name="Bt")
        nc.sync.dma_start(out=At[:], in_=rap(cost_pad, (128 * t + 1) * CS + CO - 1, [[CS, 128], [1, 513]]))
        nc.sync.dma_start(out=Bt[:], in_=rap(cost_pad, (128 * t) * CS + CO - 1, [[CS, 128], [1, 513]]))
        m1 = wk.tile([128, 512], F32, tag="t512", name="m1")
        v.tensor_tensor(out=m1[:], in0=Bt[:, 0:512], in1=At[:, 0:512], op=OP.min)
        v.tensor_tensor(out=m1[:], in0=m1[:], in1=Bt[:, 1:513], op=OP.min)
        e0 = wk.tile([128, 512], F32, tag="t512", name="e0")
        v.tensor_tensor(out=e0[:], in0=Bt[:, 0:512], in1=m1[:], op=OP.is_equal)
        t1 = wk.tile([128, 512], F32, tag="t512", name="t1")
        v.tensor_tensor(out=t1[:], in0=At[:, 0:512], in1=m1[:], op=OP.is_equal)
        v.tensor_scalar(out=e0[:], in0=e0[:], scalar1=-1.0, scalar2=1.0, op0=OP.mult, op1=OP.add)
        v.tensor_scalar(out=t1[:], in0=t1[:], scalar1=-1.0, scalar2=2.0, op0=OP.mult, op1=OP.add)
        ct = wk.tile([128, 512], F32, tag="t512", name="ct")
        v.tensor_tensor(out=ct[:], in0=e0[:], in1=t1[:], op=OP.mult)
        nc.sync.dma_start(out=rap(c_pad, (128 * t + 1) * CPC, [[CPC, 128], [1, 512]]), in_=ct[:])

    pad3 = wk.tile([128, 520], F32, tag="t512", name="pad3")
    v.memset(pad3[:], 3.0)
    nc.sync.dma_start(out=rap(c_pad, 513 * CPC, [[1, 520]]), in_=pad3[0:1, 0:520])
    nc.sync.dma_start(out=rap(c_pad, 512, [[CPC, 515], [1, 1]]), in_=pad3[0:1, 0:515])

    for t in range(4):
        Cs = wk.tile([128, 513], F32, tag="t512", name="Cs")
        Cc = wk.tile([128, 513], F32, tag="t512", name="Cc")
        nc.sync.dma_start(out=Cs[:], in_=rap(c_pad, (128 * t + 2) * CPC, [[CPC, 128], [1, 513]]))
        nc.sync.dma_start(out=Cc[:], in_=rap(c_pad, (128 * t + 1) * CPC + 1, [[CPC, 128], [1, 513]]))
        e0s = wk.tile([128, 512], F32, tag="t512", name="e0s")
        v.tensor_scalar(out=e0s[:], in0=Cs[:, 1:513], scalar1=0.0, scalar2=None, op0=OP.is_equal)
        e2s = wk.tile([128, 512], F32, tag="t512", name="e2s")
        v.tensor_scalar(out=e2s[:], in0=Cs[:, 0:512], scalar1=2.0, scalar2=None, op0=OP.is_equal)
        c1s = wk.tile([128, 512], F32, tag="t512", name="c1s")
        v.tensor_scalar(out=c1s[:], in0=Cc[:, 0:512], scalar1=1.0, scalar2=None, op0=OP.is_equal)
        sI = wk.tile([128, 1], F32, tag="tiny", name="sI")
        v.tensor_scalar(out=sI[:], in0=if_[:], scalar1=float(128 * t), scalar2=None, op0=OP.add)
        v.tensor_tensor(out=sI[:], in0=sI[:], in1=lbc[:], op=OP.is_equal)
        sd = wk.tile([128, 512], F32, tag="t512", name="sd")
        v.tensor_scalar(out=sd[:], in0=jf[:], scalar1=lbc[:, 0:1], scalar2=None, op0=OP.is_equal)
        v.tensor_scalar(out=sd[:], in0=sd[:], scalar1=sI[:, 0:1], scalar2=None, op0=OP.mult)
        for tile_, padd in ((e0s, e0_pad), (e2s, e2_pad), (c1s, c1_pad), (sd, sd_pad)):
            nc.sync.dma_start(out=rap(padd, 128 * t * EC + CO_E, [[EC, 128], [1, 512]]), in_=tile_[:])

    # validity mask M[p, mb] = 1 iff 508 <= mb + 4p <= 571
    Ti = pool.tile([128, NSTEP + 4], I32, tag="Ti")
    gp.iota(Ti[:], pattern=[[1, NSTEP + 4]], base=0, channel_multiplier=4)
    Tf = pool.tile([128, NSTEP + 4], F32, tag="Tf")
    v.tensor_copy(out=Tf[:], in_=Ti[:])
    Ma = wk.tile([128, NSTEP + 4], F32, tag="Ma")
    v.tensor_scalar(out=Ma[:], in0=Tf[:], scalar1=507.5, scalar2=None, op0=OP.is_ge)
    Mv = pool.tile([128, NSTEP + 4], F32, tag="Mv")
    v.tensor_scalar(out=Mv[:], in0=Tf[:], scalar1=571.5, scalar2=None, op0=OP.is_le)
    v.tensor_tensor(out=Mv[:], in0=Mv[:], in1=Ma[:], op=OP.mult)

    if STOP_AFTER == 'bulk':
        for c_ in reversed(nc._dtw_pool_cms):
            c_.__exit__(None, None, None)
        return
    # ---------------- backward wavefront ----------------
    E0g = [pool.tile([128, ROUND * W], F32, tag=f"e0g{g}", name=f"e0g{g}") for g in range(2)]
    E2g = [pool.tile([128, ROUND * W], F32, tag=f"e2g{g}", name=f"e2g{g}") for g in range(2)]
    # C1g slots are U=33 wide: data [0:32), col32 = 1.0 (static) — reversed
    # scan's first element is (1.0*init) max 0.0 = init, writing the guard.
    C1g = [pool.tile([128, ROUND * U], F32, tag=f"c1g{g}", name=f"c1g{g}") for g in range(2)]
    SDg = [pool.tile([128, ROUND * W], F32, tag=f"sdg{g}", name=f"sdg{g}") for g in range(2)]
    Pg = [pool.tile([128, GW], F32, tag=f"pg{g}", name=f"pg{g}") for g in range(2)]
    v.memset(C1g[0][:], 1.0)
    v.memset(C1g[1][:], 1.0)
    e4x = pool.tile([128, U], F32, tag="e4x")
    v.memset(e4x[:], 0.0)
    e6s = pool.tile([128, W], F32, tag="e6s")
    praw = pool.tile([128, U], F32, tag="praw")
    v.memset(Pg[0][:], 0.0)
    v.memset(Pg[1][:], 0.0)

    def p_slice(mb, c0, c1):
        g = (mb // ROUND) % 2
        s0 = (mb % ROUND) * U
        return Pg[g][:, s0 + c0:s0 + c1]

    def ew_slice(Wg, mb):
        g = (mb // ROUND) % 2
        s0 = (mb % ROUND) * W
        return Wg[g][:, s0:s0 + W]

    def c1_slice(mb):
        g = (mb // ROUND) % 2
        s0 = (mb % ROUND) * U
        return C1g[g][:, s0:s0 + U]

    def bfill(R, padd, Wg, sw=W):
        # addr = p*(4EC - W) + (3-b)*EC + (142-4R-a)*W + f + CO_E,  t = 4a + b
        g = R % 2
        for b in range(4):
            src = rap(padd, (3 - b) * EC + (142 - (ROUND // 4) * R) * W + CO_E,
                      [[4 * EC - W, 128], [-W, ROUND // 4], [1, W]])
            dst = bass.AP(tensor=Wg[g][:].tensor, offset=Wg[g][:].offset + b * sw,
                          ap=[list(Wg[g][:].ap[0]), [4 * sw, ROUND // 4], [1, W]])
            nc.sync.dma_start(out=dst, in_=src)

    def p_deskew(R):
        g = R % 2
        for b in range(4):
            dst = rap(p_pad, (3 - b) * EC + (142 - (ROUND // 4) * R) * W + CO_E,
                      [[4 * EC - W, 128], [-W, ROUND // 4], [1, W]])
            src = bass.AP(tensor=Pg[g][:].tensor, offset=Pg[g][:].offset + b * U,
                          ap=[list(Pg[g][:].ap[0]), [4 * U, ROUND // 4], [1, W]])
            nc.sync.dma_start(out=dst, in_=src)

    SD_LAST_ROUND = 300 // ROUND  # Sd only read at mb <= 300 (L >= 256)
    for padd, Wg, sw in ((e0_pad, E0g, W), (e2_pad, E2g, W), (c1_pad, C1g, U), (sd_pad, SDg, W)):
        bfill(0, padd, Wg, sw)
        bfill(1, padd, Wg, sw)
    for mb in range(NSTEP):
        if mb % 4 == 0:
            ps2 = psp.tile([128, U], F32, tag="shift")
            pe.matmul(ps2[:], eye_dn[:], p_slice(mb - 1, 0, U), start=True, stop=True)
            v.tensor_tensor(out=e4x[:, 0:W], in0=ew_slice(E0g, mb), in1=ps2[:, 1:U], op=OP.mult)
            v.tensor_tensor(out=e6s[:], in0=ew_slice(E2g, mb), in1=ps2[:, 0:W], op=OP.mult)
        else:
            v.tensor_tensor(out=e4x[:, 0:W], in0=ew_slice(E0g, mb), in1=p_slice(mb - 1, 1, U), op=OP.mult)
            gp.tensor_tensor(out=e6s[:], in0=ew_slice(E2g, mb), in1=p_slice(mb - 1, 0, W), op=OP.mult)
        v.tensor_tensor(out=e4x[:, 0:W], in0=e4x[:, 0:W], in1=e6s[:], op=OP.max)
        if mb <= 300:
            v.tensor_tensor(out=e4x[:, 0:W], in0=e4x[:, 0:W], in1=ew_slice(SDg, mb), op=OP.max)
        init = p_slice(mb - 4, 0, 1)
        v.tensor_tensor_scan(out=praw[:, ::-1], data0=c1_slice(mb)[:, ::-1],
                             data1=e4x[:, ::-1], initial=init, op0=OP.mult, op1=OP.max)
        v.tensor_scalar(out=p_slice(mb, 0, U), in0=praw[:],
                        scalar1=Mv[:, mb:mb + 1], scalar2=0.5, op0=OP.mult, op1=OP.is_ge)
        if mb % ROUND == ROUND - 1:
            p_deskew(mb // ROUND)
            if mb // ROUND + 2 < NROUND:
                for padd, Wg, sw in ((e0_pad, E0g, W), (e2_pad, E2g, W), (c1_pad, C1g, U)):
                    bfill(mb // ROUND + 2, padd, Wg, sw)
                if mb // ROUND + 2 <= SD_LAST_ROUND:
                    bfill(mb // ROUND + 2, sd_pad, SDg)
    p_deskew(NROUND - 1)

    # ---------------- path01 repack ----------------
    for t in range(4):
        pt = wk.tile([128, 512], F32, tag="t512", name="pt")
        nc.sync.dma_start(out=pt[:], in_=rap(p_pad, 128 * t * EC + CO_E, [[EC, 128], [1, 512]]))
        nc.sync.dma_start(out=path_out[128 * t:128 * (t + 1), :], in_=pt[:])
    if DEBUG:
        for t in range(4):
            ctd = wk.tile([128, 512], F32, tag="t512", name="ctd")
            nc.sync.dma_start(out=ctd[:], in_=rap(cost_pad, (128 * t + 1) * CS + CO, [[CS, 128], [1, 512]]))
            nc.sync.dma_start(out=dbg['cost'][128 * t:128 * (t + 1), :], in_=ctd[:])
            ccd = wk.tile([128, 512], F32, tag="t512", name="ccd")
            nc.sync.dma_start(out=ccd[:], in_=rap(c_pad, (128 * t + 1) * CPC, [[CPC, 128], [1, 512]]))
            nc.sync.dma_start(out=dbg['C'][128 * t:128 * (t + 1), :], in_=ccd[:])
    if getattr(nc, '_dtw_cc_cm', None) is not None:
        nc._dtw_cc_cm.__exit__(None, None, None)
    for c_ in reversed(nc._dtw_pool_cms):
        c_.__exit__(None, None, None)


def make_host_inputs(vec_b, music_b, rl_b, weights):
    """Per-core in_map dict from one sample's data. weights: dict of full arrays."""
    m = {
        'vec': np.ascontiguousarray(vec_b, np.float32),
        'music': np.ascontiguousarray(music_b, np.float32),
        'rl': np.asarray([rl_b], np.int32),
        'eye_up': np.eye(128, k=1).astype(np.float32),
        'eye_dn': np.eye(128, k=-1).astype(np.float32),
        'ones_bc': np.ones((1, 128), np.float32),
    }
    for nm in ('q1', 'q2', 'q3', 'k1', 'k2', 'k3'):
        w = weights['w' + nm]  # [Cout, Cin, 3]
        m['w' + nm] = np.ascontiguousarray(w.transpose(2, 1, 0), np.float32)  # [3, Cin, Cout]
        m['b' + nm] = np.ascontiguousarray(weights['b' + nm].reshape(-1, 1), np.float32)
    return m


# ---------------------------------------------------------------- host entry
_CACHED = {}


def _get_nc():
    if 'nc' not in _CACHED:
        _CACHED['nc'] = build_program(n_cores=8, with_collective=True)
    return _CACHED['nc']


def kernel(vec, music, real_length, qw1, qb1, qw2, qb2, qw3, qb3,
           kw1, kb1, kw2, kb2, kw3, kb3):
    from concourse.bass_utils import run_bass_kernel_spmd
    weights = {'wq1': qw1, 'bq1': qb1, 'wq2': qw2, 'bq2': qb2,
               'wq3': qw3, 'bq3': qb3, 'wk1': kw1, 'bk1': kb1,
               'wk2': kw2, 'bk2': kb2, 'wk3': kw3, 'bk3': kb3}
    weights = {k: np.asarray(v_, np.float32) for k, v_ in weights.items()}
    vec = np.asarray(vec, np.float32)
    music = np.asarray(music, np.float32)
    rl = np.asarray(real_length, np.int32)
    nc = _get_nc()
    in_maps = [make_host_inputs(vec[b], music[b], rl[b], weights) for b in range(8)]
    res = run_bass_kernel_spmd(nc, in_maps, list(range(8)))
    path01 = np.stack([res.results[c]['path01'] for c in range(8)]).astype(np.float32)
    dist = np.stack([res.results[c]['dist'] for c in range(8)]).astype(np.float32)
    return (path01, dist)



# revision 10
# speedup vs baseline: 54.5689x; 49.9510x over previous
"""Bass/Tile program for nn_DTWModel on TRN2: conv encoders + euclidean dist
+ global min-max norm + exact DTW (forward wavefront row-scans, bulk choice
extraction, backward path-marking wavefront).

Layout summary (per core, one sample):
- rows r=0..511 of the DTW matrix; partition p owns rows 4p..4p+3.
- forward: unit (k,s) = (row 4p+k, col-strip s of width W=32) processed at
  step m = 4p + 4s + k.  All partitions share step-uniform APs via a
  32-slot rotating window (2 group tiles of 16 slots); slot = m % 32.
- CB slot layout: [guard][32 cost values]; guard(slot m) = last value of
  slot m-4 (same row, previous strip) = cost[r][s*W-1].
- scan: state = (u min state) + d  == min(min(pd,up),left)+d of reference.
- u = min(CB(m-1)[0:32], CB(m-1)[1:33]) = min(pd, up) from row r-1.
- k=0 rows need row 4p-1 from partition p-1: PE matmul with shifted
  identity moves the slot down one partition (psum[p] = slot[p-1]).
- cost deskewed to DRAM via p-linear strided DMAs every 16 steps.
- bulk phase recomputes choices C from cost with reference tie-break, then
  static masks E0s/E2s/c1s and seed Sd, all written to padded DRAM.
- backward: P[i][j] = max(Sd, E0s*P[i+1][j+1], E2s*P[i+1][j], c1s-scan)
  processed as mirrored wavefront with reversed ttscan; P masked NaN-proof
  by validity mask M via (P*M) is_ge 0.5.
"""
import sys as _sys
if '/opt/trn_rl_repo' not in _sys.path:
    _sys.path.insert(0, '/opt/trn_rl_repo')
import numpy as np
import concourse.bass as bass
import concourse.mybir as mybir
from concourse.vector_clock import ScopedClock
from concourse.tile import TileContext

F32 = mybir.dt.float32
I32 = mybir.dt.int32
OP = mybir.AluOpType
ACT = mybir.ActivationFunctionType
AX = mybir.AxisListType

LARGE = float(np.float32(1e30))
SLOPE = float(np.float32(0.2))
DEBUG = False
STOP_AFTER = None  # 'front'|'fwd'|'bulk'|None

W = 32          # strip width
U = 33          # slot width (guard + W)
S = 512 // W    # strips per row = 16
NSTEP = 4 * 127 + 4 * (S - 1) + 3 + 1   # 572 steps, m in [0, 572)
ROUND = 32
NROUND = (NSTEP + ROUND - 1) // ROUND
GW = ROUND * U  # group tile width = 528

# cost_pad DRAM layout
CS = 4672       # row stride (cols)
CO = 4064       # data col offset; col CO-1 = INF guard (j=-1)
CROWS = 514     # row i stored at row i+1; row 0 = INF

# C_pad layout: row r stored at r+1; rows 0 unused, row 513 = 3.0 (virtual r=512)
CPR, CPC = 515, 520

# E/Sd/P pads
EC = 8672
CO_E = 4096
EROWS = 512


class SplitDrainTileContext(TileContext):
    """Final drain must carry <=1 sem wait for this neuronxcc."""

    def _drain_and_barrier(self, tick_clock, wait_clock):
        drain_inst = self.nc.sync.drain()
        wait_clock.add_sem_waits(
            drain_inst.ins, ScopedClock({None: tick_clock.global_clock})
        )
        si = drain_inst.ins.sync_info
        waits = list(si.on_wait or [])
        if len(waits) > 1:
            si.on_wait[:] = waits[:1]
            for w_ in waits[1:]:
                nop = self.nc.sync.nop(nofuse=True, hint="split_drain_wait")
                nsi = nop.ins.sync_info
                if nsi is None:
                    nop.ins.sync_info = mybir.SyncInfo(on_wait=[w_], on_update=[])
                else:
                    nsi.on_wait.append(w_)
        self.nc.all_engine_barrier()
        assert self.sems is not None
        popped = self.nc._tile_sem_poison_stack.pop()
        assert popped is self._sem_poison
        self.nc.clear_and_free_semaphores(list(self.sems.allocated().values()))
        self.nc.all_engine_barrier()


def rap(t, offset, ap):
    return bass.AP(tensor=t[:].tensor, offset=int(offset), ap=[[int(a), int(b)] for a, b in ap])


def build_program(n_cores=8, with_collective=True):
    nc = bass.Bass("TRN2", target_bir_lowering=False, debug=False,
                   num_devices=n_cores)

    # ---------------- dram tensors ----------------
    din = {}
    din['vec'] = nc.dram_tensor("vec", [126, 512], F32, kind="ExternalInput")
    din['music'] = nc.dram_tensor("music", [80, 512], F32, kind="ExternalInput")
    din['rl'] = nc.dram_tensor("rl", [1], I32, kind="ExternalInput")
    wspec = [('q1', 126, 126), ('q2', 126, 128), ('q3', 128, 128),
             ('k1', 80, 80), ('k2', 80, 128), ('k3', 128, 128)]
    for nm, ci, co in wspec:
        din['w' + nm] = nc.dram_tensor("w" + nm, [3, ci, co], F32, kind="ExternalInput")
        din['b' + nm] = nc.dram_tensor("b" + nm, [co, 1], F32, kind="ExternalInput")
    din['eye_up'] = nc.dram_tensor("eye_up", [128, 128], F32, kind="ExternalInput")
    din['eye_dn'] = nc.dram_tensor("eye_dn", [128, 128], F32, kind="ExternalInput")
    din['ones_bc'] = nc.dram_tensor("ones_bc", [1, 128], F32, kind="ExternalInput")

    dist_out = nc.dram_tensor("dist", [512, 512], F32, kind="ExternalOutput")
    path_out = nc.dram_tensor("path01", [512, 512], F32, kind="ExternalOutput")

    cost_pad = nc.dram_tensor("cost_pad", [CROWS * CS], F32)
    c_pad = nc.dram_tensor("c_pad", [CPR * CPC], F32)
    e0_pad = nc.dram_tensor("e0_pad", [EROWS * EC], F32)
    e2_pad = nc.dram_tensor("e2_pad", [EROWS * EC], F32)
    c1_pad = nc.dram_tensor("c1_pad", [EROWS * EC], F32)
    sd_pad = nc.dram_tensor("sd_pad", [EROWS * EC], F32)
    p_pad = nc.dram_tensor("p_pad", [EROWS * EC], F32)
    d_stage = nc.dram_tensor("d_stage", [524 * 512], F32)

    dbg = {}
    if DEBUG:
        dbg['qlat'] = nc.dram_tensor("dbg_qlat", [128, 512], F32, kind="ExternalOutput")
        dbg['klat'] = nc.dram_tensor("dbg_klat", [128, 512], F32, kind="ExternalOutput")
        dbg['cost'] = nc.dram_tensor("dbg_cost", [512, 512], F32, kind="ExternalOutput")
        dbg['C'] = nc.dram_tensor("dbg_C", [512, 512], F32, kind="ExternalOutput")

    with SplitDrainTileContext(nc) as tc:
        _build_body(nc, tc, din, dist_out, path_out, cost_pad, c_pad,
                    e0_pad, e2_pad, c1_pad, sd_pad, p_pad, d_stage,
                    with_collective, n_cores, dbg)
    _split_multi_waits(nc)
    return nc


def _split_multi_waits(nc, max_waits=1):
    """This neuronxcc rejects instructions with more than ~1-2 sync waits.
    Move extra waits onto same-engine NoOps inserted just before."""
    import bass_rust as _br
    ctr = [0]
    for f in nc.m.functions:
        for bb in f.blocks:
            newlist = []
            for inst in bb.instructions:
                si = inst.sync_info
                waits = list(si.on_wait) if (si and si.on_wait) else []
                if len(waits) > max_waits:
                    keep = waits[:max_waits]
                    extra = waits[max_waits:]
                    si.on_wait[:] = keep
                    for w_ in extra:
                        ctr[0] += 1
                        nop = _br.InstNoOp(name=f"waitsplit_{ctr[0]}")
                        nop.engine = inst.engine
                        nop.sync_info = mybir.SyncInfo(on_wait=[w_], on_update=[])
                        nc.register_instruction(nop, overwrite=True)
                        newlist.append(nop)
                newlist.append(inst)
            if ctr[0]:
                bb.instructions[:] = newlist
    return ctr[0]


def _build_body(nc, tc, din, dist_out, path_out, cost_pad, c_pad,
                e0_pad, e2_pad, c1_pad, sd_pad, p_pad, d_stage, with_collective,
                n_cores, dbg):
    v = nc.vector
    sc = nc.scalar
    gp = nc.gpsimd
    pe = nc.tensor

    _cms = [tc.tile_pool(name="main", bufs=1), tc.tile_pool(name="work", bufs=9),
            tc.tile_pool(name="psum", bufs=2, space="PSUM"),
            tc.tile_pool(name="psumd", bufs=2, space="PSUM")]
    pool, wk, psp, psd = [c.__enter__() for c in _cms]
    nc._dtw_pool_cms = _cms  # keep referenced; released at program end

    # ---------------- conv encoders ----------------
    def conv_chain(src_dram, cin0, chain):
        xp = pool.tile([128, 514], F32, tag=f"xpin{chain[0][0]}")
        nc.sync.dma_start(out=xp[0:cin0, 1:513], in_=din[src_dram][:])
        v.tensor_copy(out=xp[0:cin0, 0:1], in_=xp[0:cin0, 2:3])
        v.tensor_copy(out=xp[0:cin0, 513:514], in_=xp[0:cin0, 511:512])
        cur, ccur = xp, cin0
        for nm, ci, co in chain:
            wt = wk.tile([128, 3 * co], F32, tag="t512", name="wt")
            nc.sync.dma_start(out=wt[0:ci, :], in_=rap(din['w' + nm], 0, [[co, ci], [ci * co, 3], [1, co]]))
            bt = wk.tile([128, 1], F32, tag="tiny", name="bt")
            nc.sync.dma_start(out=bt[0:co, :], in_=din['b' + nm][:])
            ps = psd.tile([128, 512], F32, tag="big512")
            for dlt in range(3):
                pe.matmul(ps[0:co, :], wt[0:ci, dlt * co:(dlt + 1) * co],
                          cur[0:ccur, dlt:dlt + 512], start=(dlt == 0), stop=(dlt == 2))
            nxt = pool.tile([128, 514], F32, tag=f"xp{nm}")
            z = wk.tile([128, 512], F32, tag="t512", name="convz")
            v.tensor_scalar(out=z[0:co, :], in0=ps[0:co, :], scalar1=bt[0:co, :],
                            scalar2=None, op0=OP.add)
            z2 = wk.tile([128, 512], F32, tag="t512", name="convz2")
            v.tensor_scalar(out=z2[0:co, :], in0=z[0:co, :], scalar1=SLOPE,
                            scalar2=None, op0=OP.mult)
            v.tensor_tensor(out=nxt[0:co, 1:513], in0=z[0:co, :], in1=z2[0:co, :], op=OP.max)
            v.tensor_copy(out=nxt[0:co, 0:1], in_=nxt[0:co, 2:3])
            v.tensor_copy(out=nxt[0:co, 513:514], in_=nxt[0:co, 511:512])
            cur, ccur = nxt, co
        return cur  # [128, 514], latent in cols 1..513

    qlat = conv_chain('vec', 126, [('q1', 126, 126), ('q2', 126, 128), ('q3', 128, 128)])
    klat = conv_chain('music', 80, [('k1', 80, 80), ('k2', 80, 128), ('k3', 128, 128)])
    if DEBUG:
        nc.sync.dma_start(out=dbg['qlat'][:], in_=qlat[:, 1:513])
        nc.sync.dma_start(out=dbg['klat'][:], in_=klat[:, 1:513])

    # ---------------- dist matrix ----------------
    # |k|^2, |q|^2 via ones-matmul; G via (-2k)^T q; dist = sqrt(max(d2,0))
    ones_sb = pool.tile([128, 128], F32, tag="ones")
    v.memset(ones_sb[:], 1.0)
    ksq = wk.tile([128, 512], F32, tag="t512", name="ksq")
    v.tensor_tensor(out=ksq[:], in0=klat[:, 1:513], in1=klat[:, 1:513], op=OP.mult)
    qsq = wk.tile([128, 512], F32, tag="t512", name="qsq")
    v.tensor_tensor(out=qsq[:], in0=qlat[:, 1:513], in1=qlat[:, 1:513], op=OP.mult)
    psn = psd.tile([128, 512], F32, tag="big512")
    pe.matmul(psn[0:1, 0:512], ones_sb[:, 0:1], ksq[:], start=True, stop=True)
    psn2 = psd.tile([128, 512], F32, tag="big512")
    pe.matmul(psn2[0:1, 0:512], ones_sb[:, 0:1], qsq[:], start=True, stop=True)
    knq = pool.tile([128, 1024], F32, tag="knq")  # row0: cols 0:512=|k|^2, 512:1024=|q|^2
    v.tensor_copy(out=knq[0:1, 0:512], in_=psn[0:1, :])
    v.tensor_copy(out=knq[0:1, 512:1024], in_=psn2[0:1, :])
    ones1 = pool.tile([128, 512], F32, tag="ones1")
    v.memset(ones1[0:1, :], 1.0)
    m2k = wk.tile([128, 512], F32, tag="t512", name="m2k")
    v.tensor_scalar(out=m2k[:], in0=klat[:, 1:513], scalar1=-2.0, scalar2=None, op0=OP.mult)

    draw = pool.tile([128, 2048], F32, tag="draw")  # 4 chunks of [128,512] raw dist
    for t in range(4):
        psd2 = psd.tile([128, 512], F32, tag="big512")
        pe.matmul(psd2[:], m2k[:, t * 128:(t + 1) * 128], qlat[:, 1:513], start=True, stop=False)
        pe.matmul(psd2[:], knq[0:1, t * 128:(t + 1) * 128], ones1[0:1, 0:512], start=False, stop=False)
        pe.matmul(psd2[:], ones1[0:1, 0:128], knq[0:1, 512:1024], start=False, stop=True)
        dsq = wk.tile([128, 512], F32, tag="t512", name="dsq")
        v.tensor_scalar(out=dsq[:], in0=psd2[:], scalar1=0.0, scalar2=None, op0=OP.max)
        sc.activation(draw[:, t * 512:(t + 1) * 512], dsq[:], ACT.Sqrt)

    # min/max reduce
    red = wk.tile([128, 8], F32, tag="tiny", name="red")
    for t in range(4):
        v.tensor_reduce(out=red[:, t:t + 1], in_=draw[:, t * 512:(t + 1) * 512], axis=AX.X, op=OP.min)
        v.tensor_reduce(out=red[:, 4 + t:5 + t], in_=draw[:, t * 512:(t + 1) * 512], axis=AX.X, op=OP.max)
    red2 = wk.tile([128, 2], F32, tag="tiny", name="red2")
    v.tensor_reduce(out=red2[:, 0:1], in_=red[:, 0:4], axis=AX.X, op=OP.min)
    v.tensor_reduce(out=red2[:, 1:2], in_=red[:, 4:8], axis=AX.X, op=OP.max)
    # flatten partitions to free dim via DMA, then free reduce
    flat = pool.tile([128, 256], F32, tag="flat")
    nc.sync.dma_start(out=flat[0:1, 0:128], in_=red2[:, 0:1])
    nc.sync.dma_start(out=flat[0:1, 128:256], in_=red2[:, 1:2])
    mm = pool.tile([128, 2], F32, tag="mm")  # [1,2]: col0=-min col1=max
    v.tensor_reduce(out=mm[0:1, 0:1], in_=flat[0:1, 0:128], axis=AX.X, op=OP.min, negate=True)
    v.tensor_reduce(out=mm[0:1, 1:2], in_=flat[0:1, 128:256], axis=AX.X, op=OP.max)

    gmm = pool.tile([128, 2], F32, tag="gmm")
    if with_collective:
        _cccm = tc.tile_pool(name="ccdram", bufs=2, space="DRAM")
        nc._dtw_cc_cm = _cccm
        dramp = _cccm.__enter__()
        cc_in = dramp.tile([1, 2], F32)
        cc_out = dramp.tile([1, 2], F32)
        gp.dma_start(out=cc_in[:], in_=mm[0:1, 0:2])
        gp.collective_compute("AllReduce", OP.max,
                              replica_groups=[list(range(n_cores))],
                              ins=[cc_in.opt()], outs=[cc_out.opt()])
        gp.dma_start(out=gmm[0:1, 0:2], in_=cc_out[:])
    else:
        v.tensor_copy(out=gmm[0:1, 0:2], in_=mm[0:1, 0:2])

    # scale = 1/(max - min) = 1/(gmm[1] + gmm[0])  (gmm[0] = -min)
    sci = pool.tile([128, 2], F32, tag="sci")  # [1,1]: col0 = -min, col1 = scale
    v.tensor_copy(out=sci[0:1, 0:1], in_=gmm[0:1, 0:1])
    rngt = wk.tile([128, 1], F32, tag="tiny", name="rngt")
    v.tensor_tensor(out=rngt[0:1, :], in0=gmm[0:1, 1:2], in1=gmm[0:1, 0:1], op=OP.add)
    v.reciprocal(out=sci[0:1, 1:2], in_=rngt[0:1, :])
    # broadcast [1,2] -> [128,2] via ones matmul
    psb = psp.tile([128, 2], F32, tag="bc")
    pe.matmul(psb[:], ones_sb[0:1, :], sci[0:1, 0:2], start=True, stop=True)
    nmsc = pool.tile([128, 2], F32, tag="nmsc")
    v.tensor_copy(out=nmsc[:], in_=psb[:])

    # normalize and write dist out (+ padded staging copy for wavefront fills)
    zz = wk.tile([128, 32], F32, tag="t33", name="zz")
    v.memset(zz[:], 0.0)
    nc.sync.dma_start(out=rap(d_stage, 512 * 512, [[32, 128], [1, 32]]), in_=zz[:])
    for t in range(4):
        dn = wk.tile([128, 512], F32, tag="t512", name="dn")
        v.tensor_scalar(out=dn[:], in0=draw[:, t * 512:(t + 1) * 512],
                        scalar1=nmsc[:, 0:1], scalar2=nmsc[:, 1:2],
                        op0=OP.add, op1=OP.mult)
        nc.sync.dma_start(out=dist_out[t * 128:(t + 1) * 128, :], in_=dn[:])
        nc.sync.dma_start(out=rap(d_stage, t * 128 * 512, [[512, 128], [1, 512]]), in_=dn[:])

    # L - 1 broadcast (fp32)
    rl_sb = pool.tile([128, 2], F32, tag="rl")
    rli = wk.tile([128, 1], I32, tag="tinyi", name="rli")
    nc.sync.dma_start(out=rli[0:1, :], in_=din['rl'][:])
    v.tensor_copy(out=rl_sb[0:1, 0:1], in_=rli[0:1, :])   # int -> fp32 convert
    v.tensor_scalar(out=rl_sb[0:1, 1:2], in0=rl_sb[0:1, 0:1], scalar1=-1.0, scalar2=None, op0=OP.add)
    psb2 = psp.tile([128, 1], F32, tag="bc")
    pe.matmul(psb2[:], ones_sb[0:1, :], rl_sb[0:1, 1:2], start=True, stop=True)
    lbc = pool.tile([128, 1], F32, tag="lbc")
    v.tensor_copy(out=lbc[:], in_=psb2[:])

    # INF guards in cost_pad: row 0 (i=-1) data cols + guard col CO-1 all rows
    inf_t = pool.tile([128, 520], F32, tag="inf")
    v.memset(inf_t[:], LARGE)
    nc.sync.dma_start(out=rap(cost_pad, CO - 1, [[1, 514]]), in_=inf_t[0:1, 0:514])
    nc.sync.dma_start(out=rap(cost_pad, CS + CO - 1, [[CS, 513], [1, 1]]), in_=inf_t[0:1, 0:513])

    if STOP_AFTER == 'front':
        if getattr(nc, '_dtw_cc_cm', None) is not None:
            nc._dtw_cc_cm.__exit__(None, None, None)
        for c_ in reversed(nc._dtw_pool_cms):
            c_.__exit__(None, None, None)
        return
    inf11 = pool.tile([128, 1], F32, tag="inf11")
    v.memset(inf11[0:1, :], LARGE)
    eye_up = pool.tile([128, 128], F32, tag="eyeu")
    nc.sync.dma_start(out=eye_up[:], in_=din['eye_up'][:])
    eye_dn = pool.tile([128, 128], F32, tag="eyed")
    nc.sync.dma_start(out=eye_dn[:], in_=din['eye_dn'][:])

    # ---------------- forward wavefront ----------------
    CBg = [pool.tile([128, GW], F32, tag=f"cbg{g}", name=f"cbg{g}") for g in range(2)]
    # DWg slots are U=33 wide: col0 = 0.0 (static), data in [1:33].  The scan
    # runs 33 wide with data0 col0 = LARGE (static in utT) and data1 col0 = 0,
    # so out[0] = min(init, LARGE) + 0 = init — the guard column — fusing the
    # per-step guard copy into the scan.
    DWg = [pool.tile([128, ROUND * U], F32, tag=f"dwg{g}", name=f"dwg{g}") for g in range(2)]
    v.memset(DWg[0][:], 0.0)
    v.memset(DWg[1][:], 0.0)
    utT = pool.tile([128, U], F32, tag="utT")
    v.memset(utT[:], LARGE)
    v.memset(CBg[0][:], LARGE)
    v.memset(CBg[1][:], LARGE)
    # prime p0 row-0 cumsum start: initial of m=0 reads slot 28 (group1 slot 12) col 32 -> 0.0
    _pslot = (-4) % ROUND
    v.memset(CBg[1][0:1, _pslot * U + 32:_pslot * U + 33], 0.0)

    def cb_slice(m, c0, c1):
        g = (m // ROUND) % 2
        s0 = (m % ROUND) * U
        return CBg[g][:, s0 + c0:s0 + c1]

    def dw_slice(m):
        g = (m // ROUND) % 2
        s0 = (m % ROUND) * U
        return DWg[g][:, s0:s0 + U]

    def dfill(R):
        # dist[4p + t2][(4R + t1 - p)*W + f],  t = 4*t1 + t2; data to slot cols [1:33]
        g = R % 2
        for t2 in range(4):
            src = rap(d_stage, (ROUND // 4) * R * W + t2 * 512,
                      [[4 * 512 - W, 128], [W, ROUND // 4], [1, W]])
            dst = bass.AP(tensor=DWg[g][:].tensor, offset=DWg[g][:].offset + t2 * U + 1,
                          ap=[list(DWg[g][:].ap[0]), [4 * U, ROUND // 4], [1, W]])
            nc.sync.dma_start(out=dst, in_=src)

    def cost_deskew(R):
        g = R % 2
        for t2 in range(4):
            dst = rap(cost_pad, CS + CO + (ROUND // 4) * R * W + t2 * CS,
                      [[4 * CS - W, 128], [W, ROUND // 4], [1, W]])
            src = bass.AP(tensor=CBg[g][:].tensor, offset=CBg[g][:].offset + 1 + t2 * U,
                          ap=[list(CBg[g][:].ap[0]), [4 * U, ROUND // 4], [1, W]])
            nc.sync.dma_start(out=dst, in_=src)

    dfill(0)
    dfill(1)
    for m in range(NSTEP):
        if m % 4 == 0:
            ps = psp.tile([128, U], F32, tag="shift")
            pe.matmul(ps[:], eye_up[:], cb_slice(m - 1, 0, U), start=True, stop=True)
            scr = wk.tile([128, U], F32, tag="t33", name="scr")
            v.tensor_copy(out=scr[:], in_=ps[:])
            v.tensor_tensor(out=utT[:, 1:U], in0=scr[:, 0:W], in1=scr[:, 1:U], op=OP.min)
            if m <= 60:
                v.memset(utT[0:1, 1:U], LARGE)
        else:
            v.tensor_tensor(out=utT[:, 1:U], in0=cb_slice(m - 1, 0, W), in1=cb_slice(m - 1, 1, U), op=OP.min)
        init = cb_slice(m - 4, U - 1, U)
        v.tensor_tensor_scan(out=cb_slice(m, 0, U), data0=utT[:], data1=dw_slice(m),
                             initial=init, op0=OP.min, op1=OP.add)
        if m < 4:
            # guard col of the first 4 slots must stay LARGE (col -1 = INF),
            # not the scan-written init (partition 0 slot 0 init is 0.0)
            v.memset(cb_slice(m, 0, 1), LARGE)
        if m % ROUND == ROUND - 1:
            cost_deskew(m // ROUND)
            if m // ROUND + 2 < NROUND:
                dfill(m // ROUND + 2)
    cost_deskew(NROUND - 1)
    if STOP_AFTER == 'fwd':
        for c_ in reversed(nc._dtw_pool_cms):
            c_.__exit__(None, None, None)
        return

    # ---------------- bulk choice extraction ----------------
    iotaJ = pool.tile([128, 512], I32, tag="iJ")
    gp.iota(iotaJ[:], pattern=[[1, 512]], base=0, channel_multiplier=0)
    jf = pool.tile([128, 512], F32, tag="jf")
    v.tensor_copy(out=jf[:], in_=iotaJ[:])
    iotaI = pool.tile([128, 1], I32, tag="iI")
    gp.iota(iotaI[:], pattern=[[1, 1]], base=0, channel_multiplier=1)
    if_ = pool.tile([128, 1], F32, tag="if")
    v.tensor_copy(out=if_[:], in_=iotaI[:])

    for t in range(4):
        At = wk.tile([128, 513], F32, tag="t512", name="At")
        Bt = wk.tile([128, 513], F32, tag="t512", name="Bt")
        nc.sync.dma_start(out=At[:], in_=rap(cost_pad, (128 * t + 1) * CS + CO - 1, [[CS, 128], [1, 513]]))
        nc.sync.dma_start(out=Bt[:], in_=rap(cost_pad, (128 * t) * CS + CO - 1, [[CS, 128], [1, 513]]))
        m1 = wk.tile([128, 512], F32, tag="t512", name="m1")
        v.tensor_tensor(out=m1[:], in0=Bt[:, 0:512], in1=At[:, 0:512], op=OP.min)
        v.tensor_tensor(out=m1[:], in0=m1[:], in1=Bt[:, 1:513], op=OP.min)
        e0 = wk.tile([128, 512], F32, tag="t512", name="e0")
        v.tensor_tensor(out=e0[:], in0=Bt[:, 0:512], in1=m1[:], op=OP.is_equal)
        t1 = wk.tile([128, 512], F32, tag="t512", name="t1")
        v.tensor_tensor(out=t1[:], in0=At[:, 0:512], in1=m1[:], op=OP.is_equal)
        v.tensor_scalar(out=e0[:], in0=e0[:], scalar1=-1.0, scalar2=1.0, op0=OP.mult, op1=OP.add)
        v.tensor_scalar(out=t1[:], in0=t1[:], scalar1=-1.0, scalar2=2.0, op0=OP.mult, op1=OP.add)
        ct = wk.tile([128, 512], F32, tag="t512", name="ct")
        v.tensor_tensor(out=ct[:], in0=e0[:], in1=t1[:], op=OP.mult)
        nc.sync.dma_start(out=rap(c_pad, (128 * t + 1) * CPC, [[CPC, 128], [1, 512]]), in_=ct[:])

    pad3 = wk.tile([128, 520], F32, tag="t512", name="pad3")
    v.memset(pad3[:], 3.0)
    nc.sync.dma_start(out=rap(c_pad, 513 * CPC, [[1, 520]]), in_=pad3[0:1, 0:520])
    nc.sync.dma_start(out=rap(c_pad, 512, [[CPC, 515], [1, 1]]), in_=pad3[0:1, 0:515])

    for t in range(4):
        Cs = wk.tile([128, 513], F32, tag="t512", name="Cs")
        Cc = wk.tile([128, 513], F32, tag="t512", name="Cc")
        nc.sync.dma_start(out=Cs[:], in_=rap(c_pad, (128 * t + 2) * CPC, [[CPC, 128], [1, 513]]))
        nc.sync.dma_start(out=Cc[:], in_=rap(c_pad, (128 * t + 1) * CPC + 1, [[CPC, 128], [1, 513]]))
        e0s = wk.tile([128, 512], F32, tag="t512", name="e0s")
        v.tensor_scalar(out=e0s[:], in0=Cs[:, 1:513], scalar1=0.0, scalar2=None, op0=OP.is_equal)
        e2s = wk.tile([128, 512], F32, tag="t512", name="e2s")
        v.tensor_scalar(out=e2s[:], in0=Cs[:, 0:512], scalar1=2.0, scalar2=None, op0=OP.is_equal)
        c1s = wk.tile([128, 512], F32, tag="t512", name="c1s")
        v.tensor_scalar(out=c1s[:], in0=Cc[:, 0:512], scalar1=1.0, scalar2=None, op0=OP.is_equal)
        sI = wk.tile([128, 1], F32, tag="tiny", name="sI")
        v.tensor_scalar(out=sI[:], in0=if_[:], scalar1=float(128 * t), scalar2=None, op0=OP.add)
        v.tensor_tensor(out=sI[:], in0=sI[:], in1=lbc[:], op=OP.is_equal)
        sd = wk.tile([128, 512], F32, tag="t512", name="sd")
        v.tensor_scalar(out=sd[:], in0=jf[:], scalar1=lbc[:, 0:1], scalar2=None, op0=OP.is_equal)
        v.tensor_scalar(out=sd[:], in0=sd[:], scalar1=sI[:, 0:1], scalar2=None, op0=OP.mult)
        for tile_, padd in ((e0s, e0_pad), (e2s, e2_pad), (c1s, c1_pad), (sd, sd_pad)):
            nc.sync.dma_start(out=rap(padd, 128 * t * EC + CO_E, [[EC, 128], [1, 512]]), in_=tile_[:])

    # validity mask M[p, mb] = 1 iff 508 <= mb + 4p <= 571
    Ti = pool.tile([128, NSTEP + 4], I32, tag="Ti")
    gp.iota(Ti[:], pattern=[[1, NSTEP + 4]], base=0, channel_multiplier=4)
    Tf = pool.tile([128, NSTEP + 4], F32, tag="Tf")
    v.tensor_copy(out=Tf[:], in_=Ti[:])
    Ma = wk.tile([128, NSTEP + 4], F32, tag="Ma")
    v.tensor_scalar(out=Ma[:], in0=Tf[:], scalar1=507.5, scalar2=None, op0=OP.is_ge)
    Mv = pool.tile([128, NSTEP + 4], F32, tag="Mv")
    v.tensor_scalar(out=Mv[:], in0=Tf[:], scalar1=571.5, scalar2=None, op0=OP.is_le)
    v.tensor_tensor(out=Mv[:], in0=Mv[:], in1=Ma[:], op=OP.mult)

    if STOP_AFTER == 'bulk':
        for c_ in reversed(nc._dtw_pool_cms):
            c_.__exit__(None, None, None)
        return
    # ---------------- backward wavefront ----------------
    E0g = [pool.tile([128, ROUND * W], F32, tag=f"e0g{g}", name=f"e0g{g}") for g in range(2)]
    E2g = [pool.tile([128, ROUND * W], F32, tag=f"e2g{g}", name=f"e2g{g}") for g in range(2)]
    # C1g slots are U=33 wide: data [0:32), col32 = 1.0 (static) — reversed
    # scan's first element is (1.0*init) max 0.0 = init, writing the guard.
    C1g = [pool.tile([128, ROUND * U], F32, tag=f"c1g{g}", name=f"c1g{g}") for g in range(2)]
    SDg = [pool.tile([128, ROUND * W], F32, tag=f"sdg{g}", name=f"sdg{g}") for g in range(2)]
    Pg = [pool.tile([128, GW], F32, tag=f"pg{g}", name=f"pg{g}") for g in range(2)]
    v.memset(C1g[0][:], 1.0)
    v.memset(C1g[1][:], 1.0)
    e4x = pool.tile([128, U], F32, tag="e4x")
    v.memset(e4x[:], 0.0)
    e6s = pool.tile([128, W], F32, tag="e6s")
    praw = pool.tile([128, U], F32, tag="praw")
    v.memset(Pg[0][:], 0.0)
    v.memset(Pg[1][:], 0.0)

    def p_slice(mb, c0, c1):
        g = (mb // ROUND) % 2
        s0 = (mb % ROUND) * U
        return Pg[g][:, s0 + c0:s0 + c1]

    def ew_slice(Wg, mb):
        g = (mb // ROUND) % 2
        s0 = (mb % ROUND) * W
        return Wg[g][:, s0:s0 + W]

    def c1_slice(mb):
        g = (mb // ROUND) % 2
        s0 = (mb % ROUND) * U
        return C1g[g][:, s0:s0 + U]

    def bfill(R, padd, Wg, sw=W):
        # addr = p*(4EC - W) + (3-b)*EC + (142-4R-a)*W + f + CO_E,  t = 4a + b
        g = R % 2
        for b in range(4):
            src = rap(padd, (3 - b) * EC + (142 - (ROUND // 4) * R) * W + CO_E,
                      [[4 * EC - W, 128], [-W, ROUND // 4], [1, W]])
            dst = bass.AP(tensor=Wg[g][:].tensor, offset=Wg[g][:].offset + b * sw,
                          ap=[list(Wg[g][:].ap[0]), [4 * sw, ROUND // 4], [1, W]])
            nc.sync.dma_start(out=dst, in_=src)

    def p_deskew(R):
        g = R % 2
        for b in range(4):
            dst = rap(p_pad, (3 - b) * EC + (142 - (ROUND // 4) * R) * W + CO_E,
                      [[4 * EC - W, 128], [-W, ROUND // 4], [1, W]])
            src = bass.AP(tensor=Pg[g][:].tensor, offset=Pg[g][:].offset + b * U,
                          ap=[list(Pg[g][:].ap[0]), [4 * U, ROUND // 4], [1, W]])
            nc.sync.dma_start(out=dst, in_=src)

    SD_LAST_ROUND = 300 // ROUND  # Sd only read at mb <= 300 (L >= 256)
    for padd, Wg, sw in ((e0_pad, E0g, W), (e2_pad, E2g, W), (c1_pad, C1g, U), (sd_pad, SDg, W)):
        bfill(0, padd, Wg, sw)
        bfill(1, padd, Wg, sw)
    for mb in range(NSTEP):
        if mb % 4 == 0:
            ps2 = psp.tile([128, U], F32, tag="shift")
            pe.matmul(ps2[:], eye_dn[:], p_slice(mb - 1, 0, U), start=True, stop=True)
            v.tensor_tensor(out=e4x[:, 0:W], in0=ew_slice(E0g, mb), in1=ps2[:, 1:U], op=OP.mult)
            v.tensor_tensor(out=e6s[:], in0=ew_slice(E2g, mb), in1=ps2[:, 0:W], op=OP.mult)
        else:
            v.tensor_tensor(out=e4x[:, 0:W], in0=ew_slice(E0g, mb), in1=p_slice(mb - 1, 1, U), op=OP.mult)
            v.tensor_tensor(out=e6s[:], in0=ew_slice(E2g, mb), in1=p_slice(mb - 1, 0, W), op=OP.mult)
        v.tensor_tensor(out=e4x[:, 0:W], in0=e4x[:, 0:W], in1=e6s[:], op=OP.max)
        if mb <= 300:
            v.tensor_tensor(out=e4x[:, 0:W], in0=e4x[:, 0:W], in1=ew_slice(SDg, mb), op=OP.max)
        init = p_slice(mb - 4, 0, 1)
        v.tensor_tensor_scan(out=praw[:, ::-1], data0=c1_slice(mb)[:, ::-1],
                             data1=e4x[:, ::-1], initial=init, op0=OP.mult, op1=OP.max)
        v.tensor_scalar(out=p_slice(mb, 0, U), in0=praw[:],
                        scalar1=Mv[:, mb:mb + 1], scalar2=0.5, op0=OP.mult, op1=OP.is_ge)
        if mb % ROUND == ROUND - 1:
            p_deskew(mb // ROUND)
            if mb // ROUND + 2 < NROUND:
                for padd, Wg, sw in ((e0_pad, E0g, W), (e2_pad, E2g, W), (c1_pad, C1g, U)):
                    bfill(mb // ROUND + 2, padd, Wg, sw)
                if mb // ROUND + 2 <= SD_LAST_ROUND:
                    bfill(mb // ROUND + 2, sd_pad, SDg)
    p_deskew(NROUND - 1)

    # ---------------- path01 repack ----------------
    for t in range(4):
        pt = wk.tile([128, 512], F32, tag="t512", name="pt")
        nc.sync.dma_start(out=pt[:], in_=rap(p_pad, 128 * t * EC + CO_E, [[EC, 128], [1, 512]]))
        nc.sync.dma_start(out=path_out[128 * t:128 * (t + 1), :], in_=pt[:])
    if DEBUG:
        for t in range(4):
            ctd = wk.tile([128, 512], F32, tag="t512", name="ctd")
            nc.sync.dma_start(out=ctd[:], in_=rap(cost_pad, (128 * t + 1) * CS + CO, [[CS, 128], [1, 512]]))
            nc.sync.dma_start(out=dbg['cost'][128 * t:128 * (t + 1), :], in_=ctd[:])
            ccd = wk.tile([128, 512], F32, tag="t512", name="ccd")
            nc.sync.dma_start(out=ccd[:], in_=rap(c_pad, (128 * t + 1) * CPC, [[CPC, 128], [1, 512]]))
            nc.sync.dma_start(out=dbg['C'][128 * t:128 * (t + 1), :], in_=ccd[:])
    if getattr(nc, '_dtw_cc_cm', None) is not None:
        nc._dtw_cc_cm.__exit__(None, None, None)
    for c_ in reversed(nc._dtw_pool_cms):
        c_.__exit__(None, None, None)


def make_host_inputs(vec_b, music_b, rl_b, weights):
    """Per-core in_map dict from one sample's data. weights: dict of full arrays."""
    m = {
        'vec': np.ascontiguousarray(vec_b, np.float32),
        'music': np.ascontiguousarray(music_b, np.float32),
        'rl': np.asarray([rl_b], np.int32),
        'eye_up': np.eye(128, k=1).astype(np.float32),
        'eye_dn': np.eye(128, k=-1).astype(np.float32),
        'ones_bc': np.ones((1, 128), np.float32),
    }
    for nm in ('q1', 'q2', 'q3', 'k1', 'k2', 'k3'):
        w = weights['w' + nm]  # [Cout, Cin, 3]
        m['w' + nm] = np.ascontiguousarray(w.transpose(2, 1, 0), np.float32)  # [3, Cin, Cout]
        m['b' + nm] = np.ascontiguousarray(weights['b' + nm].reshape(-1, 1), np.float32)
    return m


# ---------------------------------------------------------------- host entry
_CACHED = {}


def _get_nc():
    if 'nc' not in _CACHED:
        _CACHED['nc'] = build_program(n_cores=8, with_collective=True)
    return _CACHED['nc']


def kernel(vec, music, real_length, qw1, qb1, qw2, qb2, qw3, qb3,
           kw1, kb1, kw2, kb2, kw3, kb3):
    from concourse.bass_utils import run_bass_kernel_spmd
    weights = {'wq1': qw1, 'bq1': qb1, 'wq2': qw2, 'bq2': qb2,
               'wq3': qw3, 'bq3': qb3, 'wk1': kw1, 'bk1': kb1,
               'wk2': kw2, 'bk2': kb2, 'wk3': kw3, 'bk3': kb3}
    weights = {k: np.asarray(v_, np.float32) for k, v_ in weights.items()}
    vec = np.asarray(vec, np.float32)
    music = np.asarray(music, np.float32)
    rl = np.asarray(real_length, np.int32)
    nc = _get_nc()
    in_maps = [make_host_inputs(vec[b], music[b], rl[b], weights) for b in range(8)]
    res = run_bass_kernel_spmd(nc, in_maps, list(range(8)))
    path01 = np.stack([res.results[c]['path01'] for c in range(8)]).astype(np.float32)
    dist = np.stack([res.results[c]['dist'] for c in range(8)]).astype(np.float32)
    return (path01, dist)

